# revision 10
# baseline (speedup 1.0000x reference)
# PointNet Feature Propagation kernel for Trainium2 (8 NeuronCores, SPMD).
#
# Sharding: data-parallel over batch B=8 -> 1 batch element per core.
# Per core:
#   1. negd'[n,s] = 2*x1.x2 - |x2|^2 via K=4 matmul (PE). Top-3-smallest
#      selection keys; the per-row |x1|^2 constant is added back later.
#   2. nc.vector.max (top-8 values) + max_index (indices; HW returns
#      ascending distinct indices for exact ties, matching jax top_k).
#   3. Weights w_j from recovered distances d_j = |x1|^2 - m_j.
#   4. Gather p2^T rows via gpsimd indirect DMA (3 x [128,1]-offset calls).
#   5. interp^T = sum_j g_j^T @ diag(w_j) on PE (weighted-sum + transpose).
#   6. 2-layer 1x1-conv MLP on PE; BatchNorm stats via ACT accum_out with
#      cross-core AllReduce (mean/E[x^2]); normalize+ReLU fused in ACT.
import numpy as np

import concourse.bass as bass
import concourse.bacc as bacc
import concourse.tile as tile
from concourse import mybir
from concourse.bass_utils import run_bass_kernel_spmd
from concourse.masks import make_identity

B, N, S = 8, 8192, 2048
D1, D2 = 128, 256
IN_CH = D1 + D2
OUT1, OUT2 = 256, 256
BN_EPS = 1e-5
REC_EPS = 1e-8
NT = N // 128          # 64 n-tiles
NCH = N // 512         # 16 MLP chunks
CNT = float(B * N)     # BN population
F32 = mybir.dt.float32
U32 = mybir.dt.uint32

_CACHE = {}


def _build_program():
    nc = bacc.Bacc("TRN2", target_bir_lowering=False, debug=False, num_devices=8)

    g1_in = nc.dram_tensor("g1", [NT, 4, 128], F32, kind="ExternalInput").ap()
    g2_in = nc.dram_tensor("g2", [4, S], F32, kind="ExternalInput").ap()
    x1sq_in = nc.dram_tensor("x1sq", [NT, 128, 1], F32, kind="ExternalInput").ap()
    p2t_in = nc.dram_tensor("p2t", [S, D2], F32, kind="ExternalInput").ap()
    p1_in = nc.dram_tensor("p1", [D1, N], F32, kind="ExternalInput").ap()
    w0t_in = nc.dram_tensor("w0t", [128, 3, 2, 128], F32, kind="ExternalInput").ap()
    w1t_in = nc.dram_tensor("w1t", [128, 2, 2, 128], F32, kind="ExternalInput").ap()
    # affine params: [layer, chunk, part, {gamma, beta}]
    aff_in = nc.dram_tensor("aff", [2, 2, 128, 2], F32, kind="ExternalInput").ap()
    out_ext = nc.dram_tensor("out", [OUT2, N], F32, kind="ExternalOutput").ap()
    y2_dram = nc.dram_tensor("y2stage", [2, 128, N], F32).ap()

    with tile.TileContext(nc) as tc:
        _emit(tc, nc, g1_in, g2_in, x1sq_in, p2t_in, p1_in, w0t_in, w1t_in,
              aff_in, out_ext, y2_dram)
    nc.compile()
    return nc


def _emit(tc, nc, g1_in, g2_in, x1sq_in, p2t_in, p1_in, w0t_in, w1t_in,
          aff_in, out_ext, y2_dram):
    from contextlib import ExitStack
    ctx = ExitStack()
    with ctx:
        singles = ctx.enter_context(tc.tile_pool(name="singles", bufs=1))
        dpsum = ctx.enter_context(tc.tile_pool(name="dpsum", bufs=1, space="PSUM"))
        ipsum = ctx.enter_context(tc.tile_pool(name="ipsum", bufs=2, space="PSUM"))
        mpsum = ctx.enter_context(tc.tile_pool(name="mpsum", bufs=2, space="PSUM"))
        dsb_pool = ctx.enter_context(tc.tile_pool(name="dsb", bufs=3))
        small = ctx.enter_context(tc.tile_pool(name="small", bufs=4))
        gpool = ctx.enter_context(tc.tile_pool(name="gath", bufs=3))
        xkpool = ctx.enter_context(tc.tile_pool(name="xk", bufs=3))
        p1pool = ctx.enter_context(tc.tile_pool(name="p1c", bufs=3))
        ypool = ctx.enter_context(tc.tile_pool(name="ych", bufs=3))
        dumpp = ctx.enter_context(tc.tile_pool(name="dump", bufs=2))
        dram = ctx.enter_context(tc.tile_pool(name="dramb", bufs=1, space="DRAM"))

        # ---------- resident setup ----------
        g2sb = singles.tile([4, S], F32)
        nc.sync.dma_start(g2sb[:], g2_in)
        ident = singles.tile([128, 128], F32)
        make_identity(nc, ident[:])
        w0t = singles.tile([128, 3, 2, 128], F32)
        nc.sync.dma_start(w0t[:], w0t_in)
        w1t = singles.tile([128, 2, 2, 128], F32)
        nc.sync.dma_start(w1t[:], w1t_in)
        affs = singles.tile([128, 2, 2, 2], F32)  # [part, layer, chunk, g/b]
        for l in range(2):
            for mo in range(2):
                nc.sync.dma_start(affs[:, l, mo, :], aff_in[l, mo])
        epst = singles.tile([128, 1], F32)
        nc.vector.memset(epst[:], BN_EPS)
        # y1 stays resident in SBUF: [2 chunks][128, N]
        y1sb = [singles.tile([128, N], F32, tag=f"y1sb{mo}", name=f"y1sb{mo}")
                for mo in range(2)]
        # per-chunk stat partials
        s1acc = singles.tile([128, 2, NCH], F32)   # sum(y1) [part, mo, chunk]
        s2acc = singles.tile([128, 2, NCH], F32)   # sum(y1^2)
        t1acc = singles.tile([128, 2, NCH], F32)   # sum(y2)
        t2acc = singles.tile([128, 2, NCH], F32)   # sum(y2^2)

        # ---------- phase 1: distances, top-3, gather, interp, MLP layer 1 ----------
        xk1 = xk2 = None
        for t in range(NT):
            ci, q = divmod(t, 4)
            g1t = small.tile([4, 128], F32, tag="g1t")
            nc.sync.dma_start(g1t[:], g1_in[t])
            x1sqt = small.tile([128, 1], F32, tag="x1sqt")
            nc.sync.dma_start(x1sqt[:], x1sq_in[t])

            dp = dpsum.tile([128, S], F32)
            for sc in range(S // 512):
                nc.tensor.matmul(
                    out=dp[:, sc * 512:(sc + 1) * 512],
                    lhsT=g1t[:],
                    rhs=g2sb[:, sc * 512:(sc + 1) * 512],
                    start=True, stop=True,
                )
            dsb = dsb_pool.tile([128, S], F32)
            nc.scalar.activation(out=dsb[:], in_=dp[:],
                                 func=mybir.ActivationFunctionType.Copy)

            m8 = small.tile([128, 8], F32, tag="m8")
            nc.vector.max(out=m8[:], in_=dsb[:])
            idx8 = small.tile([128, 8], U32, tag="idx8")
            nc.vector.max_index(out=idx8[:], in_max=m8[:], in_values=dsb[:])

            # d_j = x1sq - m_j + eps ; r_j = 1/d_j ; w_j = r_j / sum(r)
            d3 = small.tile([128, 3], F32, tag="d3")
            nc.gpsimd.tensor_tensor(out=d3[:], in0=x1sqt[:].to_broadcast([128, 3]),
                                    in1=m8[:, 0:3], op=mybir.AluOpType.subtract)
            nc.gpsimd.tensor_scalar_add(d3[:], d3[:], REC_EPS)
            r3 = small.tile([128, 3], F32, tag="r3")
            nc.vector.reciprocal(out=r3[:], in_=d3[:])
            rs = small.tile([128, 1], F32, tag="rs")
            nc.gpsimd.tensor_add(rs[:], r3[:, 0:1], r3[:, 1:2])
            nc.gpsimd.tensor_add(rs[:], rs[:], r3[:, 2:3])
            rsi = small.tile([128, 1], F32, tag="rsi")
            nc.vector.reciprocal(out=rsi[:], in_=rs[:])
            w3 = small.tile([128, 3], F32, tag="w3")
            nc.gpsimd.tensor_tensor(out=w3[:], in0=r3[:],
                                    in1=rsi[:].to_broadcast([128, 3]),
                                    op=mybir.AluOpType.mult)

            # gather neighbor rows and form interp^T via diag matmuls
            gj = []
            for j in range(3):
                g = gpool.tile([128, D2], F32, tag=f"g{j}")
                nc.gpsimd.indirect_dma_start(
                    out=g[:], out_offset=None, in_=p2t_in,
                    in_offset=bass.IndirectOffsetOnAxis(ap=idx8[:, j:j + 1], axis=0),
                )
                gj.append(g)
            dj = []
            for j in range(3):
                d = small.tile([128, 128], F32, tag=f"diag{j}")
                nc.gpsimd.tensor_tensor(out=d[:], in0=ident[:],
                                        in1=w3[:, j:j + 1].to_broadcast([128, 128]),
                                        op=mybir.AluOpType.mult)
                dj.append(d)

            if q == 0:
                xk1 = xkpool.tile([128, 512], F32, tag="xk1")
                xk2 = xkpool.tile([128, 512], F32, tag="xk2")
            for mo in range(2):
                it = ipsum.tile([128, 128], F32, tag="it")
                for j in range(3):
                    nc.tensor.matmul(
                        out=it[:], lhsT=gj[j][:, mo * 128:(mo + 1) * 128],
                        rhs=dj[j][:], start=(j == 0), stop=(j == 2),
                    )
                dst = xk1 if mo == 0 else xk2
                nc.scalar.activation(out=dst[:, q * 128:(q + 1) * 128], in_=it[:],
                                     func=mybir.ActivationFunctionType.Copy)

            if q == 3:
                # MLP layer 1 on chunk ci
                p1c = p1pool.tile([128, 512], F32, tag="p1c")
                nc.sync.dma_start(p1c[:], p1_in[:, ci * 512:(ci + 1) * 512])
                for mo in range(2):
                    yp = mpsum.tile([128, 512], F32, tag="yp")
                    nc.tensor.matmul(out=yp[:], lhsT=w0t[:, 0, mo, :], rhs=p1c[:],
                                     start=True, stop=False)
                    nc.tensor.matmul(out=yp[:], lhsT=w0t[:, 1, mo, :], rhs=xk1[:],
                                     start=False, stop=False)
                    nc.tensor.matmul(out=yp[:], lhsT=w0t[:, 2, mo, :], rhs=xk2[:],
                                     start=False, stop=True)
                    ysl = y1sb[mo][:, ci * 512:(ci + 1) * 512]
                    nc.scalar.activation(out=ysl, in_=yp[:],
                                         func=mybir.ActivationFunctionType.Copy,
                                         accum_out=s1acc[:, mo, ci:ci + 1])
                    dump = dumpp.tile([128, 512], F32, tag="dump")
                    nc.scalar.activation(out=dump[:], in_=ysl,
                                         func=mybir.ActivationFunctionType.Square,
                                         accum_out=s2acc[:, mo, ci:ci + 1])

        # ---------- BN1: reduce partials, AllReduce, scale/shift ----------
        scale0, shift0 = _bn_allreduce(tc, nc, singles, small, dram, epst, affs,
                                       s1acc, s2acc, layer=0)

        # ---------- phase 2: normalize+relu y1, MLP layer 2 ----------
        for ci in range(NCH):
            y1n = []
            for mo in range(2):
                yn = ypool.tile([128, 512], F32, tag=f"y1n{mo}")
                nc.scalar.activation(out=yn[:],
                                     in_=y1sb[mo][:, ci * 512:(ci + 1) * 512],
                                     func=mybir.ActivationFunctionType.Relu,
                                     bias=shift0[mo][:], scale=scale0[mo][:])
                y1n.append(yn)
            for mo in range(2):
                yp = mpsum.tile([128, 512], F32, tag="yp")
                nc.tensor.matmul(out=yp[:], lhsT=w1t[:, 0, mo, :], rhs=y1n[0][:],
                                 start=True, stop=False)
                nc.tensor.matmul(out=yp[:], lhsT=w1t[:, 1, mo, :], rhs=y1n[1][:],
                                 start=False, stop=True)
                y2c = ypool.tile([128, 512], F32, tag="y2c")
                nc.scalar.activation(out=y2c[:], in_=yp[:],
                                     func=mybir.ActivationFunctionType.Copy,
                                     accum_out=t1acc[:, mo, ci:ci + 1])
                dump = dumpp.tile([128, 512], F32, tag="dump2")
                nc.scalar.activation(out=dump[:], in_=y2c[:],
                                     func=mybir.ActivationFunctionType.Square,
                                     accum_out=t2acc[:, mo, ci:ci + 1])
                nc.sync.dma_start(y2_dram[mo, :, ci * 512:(ci + 1) * 512], y2c[:])

        # ---------- BN2 ----------
        scale1, shift1 = _bn_allreduce(tc, nc, singles, small, dram, epst, affs,
                                       t1acc, t2acc, layer=1)

        # ---------- phase 3: final normalize+relu, write out ----------
        for ci in range(NCH):
            for mo in range(2):
                y2c = ypool.tile([128, 512], F32, tag="y2r")
                nc.sync.dma_start(y2c[:], y2_dram[mo, :, ci * 512:(ci + 1) * 512])
                oc = ypool.tile([128, 512], F32, tag="oc")
                nc.scalar.activation(out=oc[:], in_=y2c[:],
                                     func=mybir.ActivationFunctionType.Relu,
                                     bias=shift1[mo][:], scale=scale1[mo][:])
                nc.sync.dma_start(
                    out_ext[mo * 128:(mo + 1) * 128, ci * 512:(ci + 1) * 512], oc[:])


def _bn_allreduce(tc, nc, singles, small, dram, epst, affs, sacc, sqacc, layer):
    """Reduce per-chunk partials, AllReduce across 8 cores, return per-chunk
    (scale, shift) [128,1] tiles implementing gamma*(y-mean)/sqrt(var+eps)+beta
    as y*scale + shift."""
    stats = singles.tile([128, 4], F32, tag=f"stats{layer}")
    for mo in range(2):
        sdump = small.tile([128, NCH], F32, tag="sdump", name=f"sdump{layer}{mo}")
        nc.scalar.activation(out=sdump[:], in_=sacc[:, mo, :],
                             func=mybir.ActivationFunctionType.Copy,
                             accum_out=stats[:, mo:mo + 1])
        qdump = small.tile([128, NCH], F32, tag="qdump", name=f"qdump{layer}{mo}")
        nc.scalar.activation(out=qdump[:], in_=sqacc[:, mo, :],
                             func=mybir.ActivationFunctionType.Copy,
                             accum_out=stats[:, 2 + mo:3 + mo])
    inb = dram.tile([128, 4], F32, tag=f"arin{layer}")
    outb = dram.tile([128, 4], F32, tag=f"arout{layer}")
    nc.gpsimd.dma_start(inb[:], stats[:])
    nc.gpsimd.collective_compute(
        "AllReduce", mybir.AluOpType.add,
        replica_groups=[list(range(8))],
        ins=[inb.opt()], outs=[outb.opt()],
    )
    gstats = singles.tile([128, 4], F32, tag=f"gstats{layer}")
    nc.gpsimd.dma_start(gstats[:], outb[:])

    scale, shift = [], []
    for mo in range(2):
        mean = singles.tile([128, 1], F32, tag=f"mean{layer}{mo}")
        nc.gpsimd.tensor_scalar_mul(mean[:], gstats[:, mo:mo + 1], 1.0 / CNT)
        var = singles.tile([128, 1], F32, tag=f"var{layer}{mo}")
        # var = E[y^2] - mean^2 = sum2/CNT - mean*mean
        nc.gpsimd.tensor_scalar_mul(var[:], gstats[:, 2 + mo:3 + mo], 1.0 / CNT)
        nc.gpsimd.tensor_add(var[:], var[:], _neg_msq(tc, nc, small, mean, layer, mo)[:])
        sd = singles.tile([128, 1], F32, tag=f"sd_{layer}{mo}")
        nc.scalar.activation(out=sd[:], in_=var[:],
                             func=mybir.ActivationFunctionType.Sqrt,
                             bias=epst[:])
        rsd = singles.tile([128, 1], F32, tag=f"rsd{layer}{mo}")
        nc.vector.reciprocal(out=rsd[:], in_=sd[:])
        sc = singles.tile([128, 1], F32, tag=f"scale{layer}{mo}")
        nc.gpsimd.tensor_mul(sc[:], affs[:, layer, mo, 0:1], rsd[:])
        sh = singles.tile([128, 1], F32, tag=f"shift{layer}{mo}")
        # shift = beta - mean*scale
        nc.gpsimd.tensor_mul(sh[:], mean[:], sc[:])
        nc.gpsimd.tensor_sub(sh[:], affs[:, layer, mo, 1:2], sh[:])
        scale.append(sc)
        shift.append(sh)
    return scale, shift


def _neg_msq(tc, nc, small, mean, layer, mo):
    t = small.tile([128, 1], mybir.dt.float32, tag=f"nmsq{layer}{mo}")
    nc.gpsimd.tensor_mul(t[:], mean[:], mean[:])
    nc.gpsimd.tensor_scalar_mul(t[:], t[:], -1.0)
    return t


def _prep_core_inputs(b, xyz1, xyz2, points1, points2, W0, W1, g0, beta0, g1, beta1):
    x1 = xyz1[b].astype(np.float32)          # [3, N]
    x2 = xyz2[b].astype(np.float32)          # [3, S]
    x1sq = (x1 * x1).sum(0).astype(np.float32)           # [N]
    x2sq = (x2 * x2).sum(0).astype(np.float32)           # [S]
    g1m = np.concatenate([x1, np.ones((1, N), np.float32)], 0)   # [4, N]
    g1m = np.ascontiguousarray(
        g1m.reshape(4, NT, 128).transpose(1, 0, 2))              # [NT, 4, 128]
    g2m = np.concatenate([2.0 * x2, -x2sq[None]], 0).astype(np.float32)  # [4, S]
    w0t = np.ascontiguousarray(
        W0.T.reshape(3, 128, 2, 128).transpose(1, 0, 2, 3)).astype(np.float32)
    w1t = np.ascontiguousarray(
        W1.T.reshape(2, 128, 2, 128).transpose(1, 0, 2, 3)).astype(np.float32)
    aff = np.stack([
        np.stack([np.stack([g0.reshape(2, 128)[c], beta0.reshape(2, 128)[c]], -1)
                  for c in range(2)]),
        np.stack([np.stack([g1.reshape(2, 128)[c], beta1.reshape(2, 128)[c]], -1)
                  for c in range(2)]),
    ]).astype(np.float32)                                   # [2, 2, 128, 2]
    return {
        "g1": g1m,
        "g2": g2m,
        "x1sq": np.ascontiguousarray(x1sq.reshape(NT, 128, 1)),
        "p2t": np.ascontiguousarray(points2[b].T).astype(np.float32),  # [S, D2]
        "p1": np.ascontiguousarray(points1[b]).astype(np.float32),     # [D1, N]
        "w0t": w0t,
        "w1t": w1t,
        "aff": aff,
    }


def kernel(xyz1, xyz2, points1, points2, W0, b0, g0, beta0, W1, b1, g1, beta1,
           **_ignored):
    # b0/b1 cancel exactly in training-mode BatchNorm (constant channel shift
    # moves y and its mean equally) so they are not used on device.
    if "nc" not in _CACHE:
        _CACHE["nc"] = _build_program()
    nc = _CACHE["nc"]
    in_maps = [
        _prep_core_inputs(b, np.asarray(xyz1), np.asarray(xyz2),
                          np.asarray(points1), np.asarray(points2),
                          np.asarray(W0), np.asarray(W1),
                          np.asarray(g0), np.asarray(beta0),
                          np.asarray(g1), np.asarray(beta1))
        for b in range(B)
    ]
    res = run_bass_kernel_spmd(nc, in_maps, list(range(8)))
    out = np.stack([res.results[c]["out"] for c in range(8)], axis=0)
    return out.astype(np.float32)


# revision 16
# speedup vs baseline: 1.0956x; 1.0956x over previous
# PointNet Feature Propagation kernel for Trainium2 (8 NeuronCores, SPMD).
#
# Sharding: data-parallel over batch B=8 -> 1 batch element per core.
# Per core:
#   1. negd'[n,s] = 2*x1.x2 - |x2|^2 via K=4 matmuls, 4 n-tiles packed into
#      the PE array concurrently with tile_position row groups (fp32).
#   2. nc.vector.max (top-8 values) + max_index (indices; HW returns
#      ascending distinct indices for exact ties, matching jax top_k).
#   3. Weights w_j from recovered distances d_j = |x1|^2 - m_j.
#   4. Gather p2^T rows via gpsimd indirect DMA (3 x [128,1]-offset calls).
#   5. interp^T = sum_j g_j^T @ diag(w_j) on PE (weighted-sum + transpose).
#   6. 2-layer 1x1-conv MLP on PE in fp32r; BatchNorm stats via ACT
#      accum_out with cross-core AllReduce; normalize+ReLU fused in ACT.
import numpy as np

import concourse.bass as bass
import concourse.bacc as bacc
import concourse.tile as tile
from concourse import mybir
from concourse.bass_utils import run_bass_kernel_spmd
from concourse.masks import make_identity

B, N, S = 8, 8192, 2048
D1, D2 = 128, 256
BN_EPS = 1e-5
REC_EPS = 1e-8
NT = N // 128          # 64 n-tiles
NSI = NT // 4          # 16 super-iterations (4 packed n-tiles each = 512 n)
NCH = N // 512         # 16 MLP chunks
CNT = float(B * N)     # BN population
F32 = mybir.dt.float32
F32R = mybir.dt.float32r
U32 = mybir.dt.uint32
AF = mybir.ActivationFunctionType
OP = mybir.AluOpType

_CACHE = {}


def _build_program():
    nc = bacc.Bacc("TRN2", target_bir_lowering=False, debug=False, num_devices=8)

    g1p_in = nc.dram_tensor("g1p", [NSI, 128, 128], F32, kind="ExternalInput").ap()
    g2r_in = nc.dram_tensor("g2r", [128, S], F32, kind="ExternalInput").ap()
    x1sq_in = nc.dram_tensor("x1sq", [NT, 128, 1], F32, kind="ExternalInput").ap()
    p2t_in = nc.dram_tensor("p2t", [S, D2], F32, kind="ExternalInput").ap()
    p1_in = nc.dram_tensor("p1", [D1, N], F32, kind="ExternalInput").ap()
    w0t_in = nc.dram_tensor("w0t", [128, 3, 2, 128], F32, kind="ExternalInput").ap()
    w1t_in = nc.dram_tensor("w1t", [128, 2, 2, 128], F32, kind="ExternalInput").ap()
    aff_in = nc.dram_tensor("aff", [2, 2, 128, 2], F32, kind="ExternalInput").ap()
    out_ext = nc.dram_tensor("out", [D2, N], F32, kind="ExternalOutput").ap()
    y2_dram = nc.dram_tensor("y2stage", [2, 128, N], F32).ap()

    with tile.TileContext(nc) as tc:
        _emit(tc, nc, g1p_in, g2r_in, x1sq_in, p2t_in, p1_in, w0t_in, w1t_in,
              aff_in, out_ext, y2_dram)
    nc.compile()
    return nc


def _emit(tc, nc, g1p_in, g2r_in, x1sq_in, p2t_in, p1_in, w0t_in, w1t_in,
          aff_in, out_ext, y2_dram):
    from contextlib import ExitStack
    ctx = ExitStack()
    with ctx:
        singles = ctx.enter_context(tc.tile_pool(name="singles", bufs=1))
        dpsum = ctx.enter_context(tc.tile_pool(name="dpsum", bufs=4, space="PSUM"))
        ipsum = ctx.enter_context(tc.tile_pool(name="ipsum", bufs=2, space="PSUM"))
        mpsum = ctx.enter_context(tc.tile_pool(name="mpsum", bufs=2, space="PSUM"))
        dsb_pool = ctx.enter_context(tc.tile_pool(name="dsb", bufs=4))
        small = ctx.enter_context(tc.tile_pool(name="small", bufs=4))
        gpool = ctx.enter_context(tc.tile_pool(name="gath", bufs=3))
        xkpool = ctx.enter_context(tc.tile_pool(name="xk", bufs=2))
        p1pool = ctx.enter_context(tc.tile_pool(name="p1c", bufs=2))
        ypool = ctx.enter_context(tc.tile_pool(name="ych", bufs=2))
        dumpp = ctx.enter_context(tc.tile_pool(name="dump", bufs=2))
        dram = ctx.enter_context(tc.tile_pool(name="dramb", bufs=1, space="DRAM"))

        # ---------- resident setup ----------
        g2r = singles.tile([128, S], F32)
        nc.sync.dma_start(g2r[:], g2r_in)
        ident = singles.tile([128, 128], F32)
        make_identity(nc, ident[:])
        w0f = singles.tile([128, 3, 2, 128], F32)
        nc.sync.dma_start(w0f[:], w0t_in)
        w1f = singles.tile([128, 2, 2, 128], F32)
        nc.sync.dma_start(w1f[:], w1t_in)
        # fp32r-rounded copies for the MLP matmuls
        w0t = singles.tile([128, 3, 2, 128], F32R)
        nc.vector.tensor_copy(w0t[:], w0f[:])
        w1t = singles.tile([128, 2, 2, 128], F32R)
        nc.vector.tensor_copy(w1t[:], w1f[:])
        affs = singles.tile([128, 2, 2, 2], F32)  # [part, layer, chunk, g/b]
        for l in range(2):
            for mo in range(2):
                nc.sync.dma_start(affs[:, l, mo, :], aff_in[l, mo])
        epst = singles.tile([128, 1], F32)
        nc.vector.memset(epst[:], BN_EPS)
        y1sb = [singles.tile([128, N], F32, tag=f"y1sb{mo}", name=f"y1sb{mo}")
                for mo in range(2)]
        s1acc = singles.tile([128, 2, NCH], F32)
        s2acc = singles.tile([128, 2, NCH], F32)
        t1acc = singles.tile([128, 2, NCH], F32)
        t2acc = singles.tile([128, 2, NCH], F32)

        # ---------- phase 1 ----------
        for si in range(NSI):
            g1sp = small.tile([128, 128], F32, tag="g1sp")
            nc.sync.dma_start(g1sp[:], g1p_in[si])

            # 4 packed n-tiles: distance matmuls + PSUM drains
            dsbs = []
            for g in range(4):
                dsb = dsb_pool.tile([128, S], F32, tag="dsb", name=f"dsb{si}_{g}")
                dsbs.append(dsb)
            for sc in range(4):
                for g in range(4):
                    dp = dpsum.tile([128, 512], F32, tag="dp", name=f"dp{si}_{sc}_{g}")
                    nc.tensor.matmul(
                        out=dp[:],
                        lhsT=g1sp[32 * g:32 * g + 4, :],
                        rhs=g2r[32 * g:32 * g + 4, sc * 512:(sc + 1) * 512],
                        start=True, stop=True, tile_position=(32 * g, 0),
                    )
                    dst = dsbs[g][:, sc * 512:(sc + 1) * 512]
                    if sc == 3:
                        nc.vector.tensor_copy(dst, dp[:])
                    else:
                        nc.scalar.activation(out=dst, in_=dp[:], func=AF.Copy)

            xk1 = xkpool.tile([128, 512], F32R, tag="xk1", name=f"xk1_{si}")
            xk2 = xkpool.tile([128, 512], F32R, tag="xk2", name=f"xk2_{si}")

            for g in range(4):
                t = 4 * si + g
                dsb = dsbs[g]
                x1sqt = small.tile([128, 1], F32, tag="x1sqt", name=f"x1sq{t}")
                nc.sync.dma_start(x1sqt[:], x1sq_in[t])

                m8 = small.tile([128, 8], F32, tag="m8", name=f"m8_{t}")
                nc.vector.max(out=m8[:], in_=dsb[:])
                idx8 = small.tile([128, 8], U32, tag="idx8", name=f"idx8_{t}")
                nc.vector.max_index(out=idx8[:], in_max=m8[:], in_values=dsb[:])

                d3 = small.tile([128, 3], F32, tag="d3", name=f"d3_{t}")
                nc.gpsimd.tensor_tensor(out=d3[:], in0=x1sqt[:].to_broadcast([128, 3]),
                                        in1=m8[:, 0:3], op=OP.subtract)
                nc.gpsimd.tensor_scalar_add(d3[:], d3[:], REC_EPS)
                r3 = small.tile([128, 3], F32, tag="r3", name=f"r3_{t}")
                nc.vector.reciprocal(out=r3[:], in_=d3[:])
                rs = small.tile([128, 1], F32, tag="rs", name=f"rs_{t}")
                nc.gpsimd.tensor_add(rs[:], r3[:, 0:1], r3[:, 1:2])
                nc.gpsimd.tensor_add(rs[:], rs[:], r3[:, 2:3])
                rsi_t = small.tile([128, 1], F32, tag="rsi", name=f"rsi_{t}")
                nc.vector.reciprocal(out=rsi_t[:], in_=rs[:])
                w3 = small.tile([128, 3], F32, tag="w3", name=f"w3_{t}")
                nc.gpsimd.tensor_tensor(out=w3[:], in0=r3[:],
                                        in1=rsi_t[:].to_broadcast([128, 3]),
                                        op=OP.mult)

                gj, dj = [], []
                for j in range(3):
                    gt = gpool.tile([128, D2], F32, tag=f"g{j}", name=f"g{t}_{j}")
                    nc.gpsimd.indirect_dma_start(
                        out=gt[:], out_offset=None, in_=p2t_in,
                        in_offset=bass.IndirectOffsetOnAxis(ap=idx8[:, j:j + 1],
                                                            axis=0),
                    )
                    gj.append(gt)
                for j in range(3):
                    d = small.tile([128, 128], F32, tag=f"diag{j}", name=f"dg{t}_{j}")
                    if j == 0:
                        nc.vector.tensor_tensor(
                            out=d[:], in0=ident[:],
                            in1=w3[:, j:j + 1].to_broadcast([128, 128]), op=OP.mult)
                    elif j == 1:
                        nc.scalar.activation(out=d[:], in_=ident[:], func=AF.Copy,
                                             scale=w3[:, j:j + 1])
                    else:
                        nc.gpsimd.tensor_tensor(
                            out=d[:], in0=ident[:],
                            in1=w3[:, j:j + 1].to_broadcast([128, 128]), op=OP.mult)
                    dj.append(d)

                for mo in range(2):
                    it = ipsum.tile([128, 128], F32, tag="it", name=f"it{t}_{mo}")
                    for j in range(3):
                        nc.tensor.matmul(
                            out=it[:], lhsT=gj[j][:, mo * 128:(mo + 1) * 128],
                            rhs=dj[j][:], start=(j == 0), stop=(j == 2),
                        )
                    dst = xk1 if mo == 0 else xk2
                    nc.scalar.activation(out=dst[:, g * 128:(g + 1) * 128], in_=it[:],
                                         func=AF.Copy)

            # MLP layer 1 on chunk ci = si (fp32r)
            ci = si
            p1c = p1pool.tile([128, 512], F32, tag="p1c", name=f"p1c{ci}")
            nc.sync.dma_start(p1c[:], p1_in[:, ci * 512:(ci + 1) * 512])
            p1r = p1pool.tile([128, 512], F32R, tag="p1r", name=f"p1r{ci}")
            nc.gpsimd.tensor_copy(p1r[:], p1c[:])
            for mo in range(2):
                yp = mpsum.tile([128, 512], F32, tag="yp", name=f"yp{ci}_{mo}")
                nc.tensor.matmul(out=yp[:], lhsT=w0t[:, 0, mo, :], rhs=p1r[:],
                                 start=True, stop=False)
                nc.tensor.matmul(out=yp[:], lhsT=w0t[:, 1, mo, :], rhs=xk1[:],
                                 start=False, stop=False)
                nc.tensor.matmul(out=yp[:], lhsT=w0t[:, 2, mo, :], rhs=xk2[:],
                                 start=False, stop=True)
                ysl = y1sb[mo][:, ci * 512:(ci + 1) * 512]
                nc.scalar.activation(out=ysl, in_=yp[:], func=AF.Copy,
                                     accum_out=s1acc[:, mo, ci:ci + 1])
                dump = dumpp.tile([128, 512], F32, tag="dump", name=f"du{ci}_{mo}")
                nc.scalar.activation(out=dump[:], in_=ysl, func=AF.Square,
                                     accum_out=s2acc[:, mo, ci:ci + 1])

        # ---------- BN1 ----------
        scale0, shift0 = _bn_allreduce(tc, nc, singles, small, dram, epst, affs,
                                       s1acc, s2acc, layer=0)

        # ---------- phase 2: normalize+relu y1 (fp32r), MLP layer 2 ----------
        for ci in range(NCH):
            y1n = []
            for mo in range(2):
                yn = ypool.tile([128, 512], F32R, tag=f"y1n{mo}", name=f"y1n{ci}_{mo}")
                src = y1sb[mo][:, ci * 512:(ci + 1) * 512]
                if mo == 0:
                    nc.scalar.activation(out=yn[:], in_=src, func=AF.Relu,
                                         bias=shift0[mo][:], scale=scale0[mo][:])
                else:
                    nc.vector.tensor_scalar(out=yn[:], in0=src,
                                            scalar1=scale0[mo][:],
                                            scalar2=shift0[mo][:],
                                            op0=OP.mult, op1=OP.add)
                    nc.vector.tensor_scalar_max(yn[:], yn[:], 0.0)
                y1n.append(yn)
            for mo in range(2):
                yp = mpsum.tile([128, 512], F32, tag="yp", name=f"y2p{ci}_{mo}")
                nc.tensor.matmul(out=yp[:], lhsT=w1t[:, 0, mo, :], rhs=y1n[0][:],
                                 start=True, stop=False)
                nc.tensor.matmul(out=yp[:], lhsT=w1t[:, 1, mo, :], rhs=y1n[1][:],
                                 start=False, stop=True)
                y2c = ypool.tile([128, 512], F32, tag="y2c", name=f"y2c{ci}_{mo}")
                nc.scalar.activation(out=y2c[:], in_=yp[:], func=AF.Copy,
                                     accum_out=t1acc[:, mo, ci:ci + 1])
                dump = dumpp.tile([128, 512], F32, tag="dump2", name=f"d2{ci}_{mo}")
                nc.scalar.activation(out=dump[:], in_=y2c[:], func=AF.Square,
                                     accum_out=t2acc[:, mo, ci:ci + 1])
                nc.sync.dma_start(y2_dram[mo, :, ci * 512:(ci + 1) * 512], y2c[:])

        # ---------- BN2 ----------
        scale1, shift1 = _bn_allreduce(tc, nc, singles, small, dram, epst, affs,
                                       t1acc, t2acc, layer=1)

        # ---------- phase 3 ----------
        for ci in range(NCH):
            for mo in range(2):
                y2c = ypool.tile([128, 512], F32, tag="y2r", name=f"y2r{ci}_{mo}")
                nc.sync.dma_start(y2c[:], y2_dram[mo, :, ci * 512:(ci + 1) * 512])
                oc = ypool.tile([128, 512], F32, tag="oc", name=f"oc{ci}_{mo}")
                if mo == 0:
                    nc.scalar.activation(out=oc[:], in_=y2c[:], func=AF.Relu,
                                         bias=shift1[mo][:], scale=scale1[mo][:])
                else:
                    nc.vector.tensor_scalar(out=oc[:], in0=y2c[:],
                                            scalar1=scale1[mo][:],
                                            scalar2=shift1[mo][:],
                                            op0=OP.mult, op1=OP.add)
                    nc.vector.tensor_scalar_max(oc[:], oc[:], 0.0)
                nc.sync.dma_start(
                    out_ext[mo * 128:(mo + 1) * 128, ci * 512:(ci + 1) * 512], oc[:])


def _bn_allreduce(tc, nc, singles, small, dram, epst, affs, sacc, sqacc, layer):
    """Reduce per-chunk partials, AllReduce over 8 cores, return per-chunk
    (scale, shift) [128,1] tiles so BN+affine is y*scale + shift."""
    stats = singles.tile([128, 4], F32, tag=f"stats{layer}", name=f"stats{layer}")
    for mo in range(2):
        sdump = small.tile([128, NCH], F32, tag="sdump", name=f"sdump{layer}{mo}")
        nc.scalar.activation(out=sdump[:], in_=sacc[:, mo, :], func=AF.Copy,
                             accum_out=stats[:, mo:mo + 1])
        qdump = small.tile([128, NCH], F32, tag="qdump", name=f"qdump{layer}{mo}")
        nc.scalar.activation(out=qdump[:], in_=sqacc[:, mo, :], func=AF.Copy,
                             accum_out=stats[:, 2 + mo:3 + mo])
    inb = dram.tile([128, 4], F32, tag=f"arin{layer}", name=f"arin{layer}")
    outb = dram.tile([128, 4], F32, tag=f"arout{layer}", name=f"arout{layer}")
    nc.gpsimd.dma_start(inb[:], stats[:])
    nc.gpsimd.collective_compute(
        "AllReduce", mybir.AluOpType.add,
        replica_groups=[list(range(8))],
        ins=[inb.opt()], outs=[outb.opt()],
    )
    gstats = singles.tile([128, 4], F32, tag=f"gstats{layer}", name=f"gstats{layer}")
    nc.gpsimd.dma_start(gstats[:], outb[:])

    scale, shift = [], []
    for mo in range(2):
        mean = singles.tile([128, 1], F32, tag=f"mean{layer}{mo}",
                            name=f"mean{layer}{mo}")
        nc.gpsimd.tensor_scalar_mul(mean[:], gstats[:, mo:mo + 1], 1.0 / CNT)
        var = singles.tile([128, 1], F32, tag=f"var{layer}{mo}",
                           name=f"var{layer}{mo}")
        nc.gpsimd.tensor_scalar_mul(var[:], gstats[:, 2 + mo:3 + mo], 1.0 / CNT)
        msq = small.tile([128, 1], F32, tag="msq", name=f"msq{layer}{mo}")
        nc.gpsimd.tensor_mul(msq[:], mean[:], mean[:])
        nc.gpsimd.tensor_sub(var[:], var[:], msq[:])
        sd = singles.tile([128, 1], F32, tag=f"sd_{layer}{mo}", name=f"sd{layer}{mo}")
        nc.scalar.activation(out=sd[:], in_=var[:], func=AF.Sqrt, bias=epst[:])
        rsd = singles.tile([128, 1], F32, tag=f"rsd{layer}{mo}",
                           name=f"rsd{layer}{mo}")
        nc.vector.reciprocal(out=rsd[:], in_=sd[:])
        sc = singles.tile([128, 1], F32, tag=f"scale{layer}{mo}",
                          name=f"scale{layer}{mo}")
        nc.gpsimd.tensor_mul(sc[:], affs[:, layer, mo, 0:1], rsd[:])
        sh = singles.tile([128, 1], F32, tag=f"shift{layer}{mo}",
                          name=f"shift{layer}{mo}")
        nc.gpsimd.tensor_mul(sh[:], mean[:], sc[:])
        nc.gpsimd.tensor_sub(sh[:], affs[:, layer, mo, 1:2], sh[:])
        scale.append(sc)
        shift.append(sh)
    return scale, shift


def _prep_core_inputs(b, xyz1, xyz2, points1, points2, W0, W1, g0, beta0, g1, beta1):
    x1 = xyz1[b].astype(np.float32)          # [3, N]
    x2 = xyz2[b].astype(np.float32)          # [3, S]
    x1sq = (x1 * x1).sum(0).astype(np.float32)
    x2sq = (x2 * x2).sum(0).astype(np.float32)
    g1m = np.concatenate([x1, np.ones((1, N), np.float32)], 0)   # [4, N]
    g1t = g1m.reshape(4, NT, 128).transpose(1, 0, 2)             # [NT, 4, 128]
    # packed layout: super-tile si holds tiles 4si+g at partition offset 32g
    g1p = np.zeros((NSI, 128, 128), np.float32)
    for g in range(4):
        g1p[:, 32 * g:32 * g + 4, :] = g1t[g::4][:NSI]           # tiles 4si+g
    # fix interleave: g1t[g::4] gives tiles g, g+4, ... = tile 4si+g for si-th ✓
    g2m = np.concatenate([2.0 * x2, -x2sq[None]], 0).astype(np.float32)  # [4, S]
    g2rep = np.zeros((128, S), np.float32)
    for g in range(4):
        g2rep[32 * g:32 * g + 4, :] = g2m
    w0t = np.ascontiguousarray(
        W0.T.reshape(3, 128, 2, 128).transpose(1, 0, 2, 3)).astype(np.float32)
    w1t = np.ascontiguousarray(
        W1.T.reshape(2, 128, 2, 128).transpose(1, 0, 2, 3)).astype(np.float32)
    aff = np.stack([
        np.stack([np.stack([g0.reshape(2, 128)[c], beta0.reshape(2, 128)[c]], -1)
                  for c in range(2)]),
        np.stack([np.stack([g1.reshape(2, 128)[c], beta1.reshape(2, 128)[c]], -1)
                  for c in range(2)]),
    ]).astype(np.float32)
    return {
        "g1p": g1p,
        "g2r": g2rep,
        "x1sq": np.ascontiguousarray(x1sq.reshape(NT, 128, 1)),
        "p2t": np.ascontiguousarray(points2[b].T).astype(np.float32),
        "p1": np.ascontiguousarray(points1[b]).astype(np.float32),
        "w0t": w0t,
        "w1t": w1t,
        "aff": aff,
    }


def kernel(xyz1, xyz2, points1, points2, W0, b0, g0, beta0, W1, b1, g1, beta1,
           **_ignored):
    # b0/b1 cancel exactly in training-mode BatchNorm (constant channel shift
    # moves y and its mean equally) so they are not used on device.
    if "nc" not in _CACHE:
        _CACHE["nc"] = _build_program()
    nc = _CACHE["nc"]
    in_maps = [
        _prep_core_inputs(b, np.asarray(xyz1), np.asarray(xyz2),
                          np.asarray(points1), np.asarray(points2),
                          np.asarray(W0), np.asarray(W1),
                          np.asarray(g0), np.asarray(beta0),
                          np.asarray(g1), np.asarray(beta1))
        for b in range(B)
    ]
    res = run_bass_kernel_spmd(nc, in_maps, list(range(8)))
    out = np.stack([res.results[c]["out"] for c in range(8)], axis=0)
    return out.astype(np.float32)


# revision 25
# speedup vs baseline: 1.2400x; 1.1318x over previous
# PointNet Feature Propagation kernel for Trainium2 (8 NeuronCores, SPMD).
#
# Sharding: data-parallel over batch B=8 -> 1 batch element per core.
# Per core:
#   1. negd'[n,s] = 2*x1.x2 - |x2|^2 via K=4 matmuls, 4 n-tiles packed into
#      the PE array concurrently with tile_position row groups (fp32).
#   2. nc.vector.max (top-8 values) + max_index (indices; HW returns
#      ascending distinct indices for exact ties, matching jax top_k).
#   3. Weights w_j from recovered distances d_j = |x1|^2 - m_j.
#   4. Gather p2^T rows via gpsimd indirect DMA (3 x [128,1]-offset calls).
#   5. interp^T = sum_j g_j^T @ diag(w_j) on PE (weighted-sum + transpose).
#   6. 2-layer 1x1-conv MLP on PE in fp32r; BatchNorm stats via ACT
#      accum_out with cross-core AllReduce; normalize+ReLU fused in ACT.
import numpy as np

import concourse.bass as bass
import concourse.bacc as bacc
import concourse.tile as tile
from concourse import mybir
from concourse.bass_utils import run_bass_kernel_spmd
from concourse.masks import make_identity

B, N, S = 8, 8192, 2048
D1, D2 = 128, 256
BN_EPS = 1e-5
REC_EPS = 1e-8
NT = N // 128          # 64 n-tiles
NSI = NT // 4          # 16 super-iterations (4 packed n-tiles each = 512 n)
NCH = N // 512         # 16 MLP chunks
CNT = float(B * N)     # BN population
F32 = mybir.dt.float32
F32R = mybir.dt.float32r
U32 = mybir.dt.uint32
AF = mybir.ActivationFunctionType
OP = mybir.AluOpType

_CACHE = {}


def _build_program():
    nc = bacc.Bacc("TRN2", target_bir_lowering=False, debug=False, num_devices=8)

    g1p_in = nc.dram_tensor("g1p", [NSI, 128, 128], F32, kind="ExternalInput").ap()
    g2r_in = nc.dram_tensor("g2r", [128, S], F32, kind="ExternalInput").ap()
    x1sq_in = nc.dram_tensor("x1sq", [NT, 128, 1], F32, kind="ExternalInput").ap()
    p2t_in = nc.dram_tensor("p2t", [S, D2], F32, kind="ExternalInput").ap()
    p1_in = nc.dram_tensor("p1", [D1, N], F32, kind="ExternalInput").ap()
    w0t_in = nc.dram_tensor("w0t", [128, 3, 2, 128], F32, kind="ExternalInput").ap()
    w1t_in = nc.dram_tensor("w1t", [128, 2, 2, 128], F32, kind="ExternalInput").ap()
    aff_in = nc.dram_tensor("aff", [2, 2, 128, 2], F32, kind="ExternalInput").ap()
    m1_in = nc.dram_tensor("m1", [128, 1], U32, kind="ExternalInput").ap()
    m2_in = nc.dram_tensor("m2", [128, 1], U32, kind="ExternalInput").ap()
    out_ext = nc.dram_tensor("out", [D2, N], F32, kind="ExternalOutput").ap()
    y2_dram = nc.dram_tensor("y2stage", [2, 128, N], F32).ap()

    with tile.TileContext(nc) as tc:
        _emit(tc, nc, g1p_in, g2r_in, x1sq_in, p2t_in, p1_in, w0t_in, w1t_in,
              aff_in, out_ext, y2_dram, m1_in, m2_in)
    nc.compile()
    return nc


def _emit(tc, nc, g1p_in, g2r_in, x1sq_in, p2t_in, p1_in, w0t_in, w1t_in,
          aff_in, out_ext, y2_dram, m1_in, m2_in):
    from contextlib import ExitStack
    ctx = ExitStack()
    with ctx:
        singles = ctx.enter_context(tc.tile_pool(name="singles", bufs=1))
        dpsum = ctx.enter_context(tc.tile_pool(name="dpsum", bufs=4, space="PSUM"))
        ipsum = ctx.enter_context(tc.tile_pool(name="ipsum", bufs=2, space="PSUM"))
        mpsum = ctx.enter_context(tc.tile_pool(name="mpsum", bufs=2, space="PSUM"))
        dsb_pool = ctx.enter_context(tc.tile_pool(name="dsb", bufs=4))
        small = ctx.enter_context(tc.tile_pool(name="small", bufs=4))
        gpool = ctx.enter_context(tc.tile_pool(name="gath", bufs=3))
        xkpool = ctx.enter_context(tc.tile_pool(name="xk", bufs=2))
        p1pool = ctx.enter_context(tc.tile_pool(name="p1c", bufs=2))
        ypool = ctx.enter_context(tc.tile_pool(name="ych", bufs=2))
        dumpp = ctx.enter_context(tc.tile_pool(name="dump", bufs=2))
        dram = ctx.enter_context(tc.tile_pool(name="dramb", bufs=1, space="DRAM"))

        # ---------- resident setup ----------
        m1t = singles.tile([128, 1], U32)
        nc.sync.dma_start(m1t[:], m1_in)
        m2t = singles.tile([128, 1], U32)
        nc.sync.dma_start(m2t[:], m2_in)
        # G2 split-fp32r prep: rows 32g+0..3 r2, +4..7 r2(dup), +8..11 e2.
        # Temporaries borrow the dsb pool's rotating [128, S] slots.
        g2raw = dsb_pool.tile([128, S], F32, tag="dsb", name="g2raw")
        nc.sync.dma_start(g2raw[:], g2r_in)
        g2q = singles.tile([128, S], F32R)
        nc.vector.tensor_copy(g2q[:], g2raw[:])
        g2dec = dsb_pool.tile([128, S], F32, tag="dsb", name="g2dec")
        nc.vector.tensor_copy(g2dec[:], g2q[:])
        g2e = dsb_pool.tile([128, S], F32, tag="dsb", name="g2e")
        nc.vector.tensor_sub(g2e[:], g2raw[:], g2dec[:])
        nc.vector.copy_predicated(g2raw[:], m2t[:].to_broadcast([128, S]), g2e[:])
        nc.vector.tensor_copy(g2q[:], g2raw[:])
        ident = singles.tile([128, 128], F32)
        make_identity(nc, ident[:])
        w0f = singles.tile([128, 3, 2, 128], F32)
        nc.sync.dma_start(w0f[:], w0t_in)
        w1f = singles.tile([128, 2, 2, 128], F32)
        nc.sync.dma_start(w1f[:], w1t_in)
        # fp32r-rounded copies for the MLP matmuls
        w0t = singles.tile([128, 3, 2, 128], F32R)
        nc.vector.tensor_copy(w0t[:], w0f[:])
        w1t = singles.tile([128, 2, 2, 128], F32R)
        nc.vector.tensor_copy(w1t[:], w1f[:])
        affs = singles.tile([128, 2, 2, 2], F32)  # [part, layer, chunk, g/b]
        for l in range(2):
            for mo in range(2):
                nc.sync.dma_start(affs[:, l, mo, :], aff_in[l, mo])
        epst = singles.tile([128, 1], F32)
        nc.vector.memset(epst[:], BN_EPS)
        y1sb = [singles.tile([128, N], F32, tag=f"y1sb{mo}", name=f"y1sb{mo}")
                for mo in range(2)]
        s1acc = singles.tile([128, 2, NCH], F32)
        s2acc = singles.tile([128, 2, NCH], F32)
        t1acc = singles.tile([128, 2, NCH], F32)
        t2acc = singles.tile([128, 2, NCH], F32)

        # ---------- phase 1 ----------
        for si in range(NSI):
            # G1 split-fp32r prep per super-tile
            g1sp = small.tile([128, 128], F32, tag="g1sp", name=f"g1sp{si}")
            nc.sync.dma_start(g1sp[:], g1p_in[si])
            g1q0 = small.tile([128, 128], F32R, tag="g1q0", name=f"g1q0{si}")
            nc.scalar.activation(out=g1q0[:], in_=g1sp[:], func=AF.Copy)
            g1dec = small.tile([128, 128], F32, tag="g1dec", name=f"g1dec{si}")
            nc.scalar.activation(out=g1dec[:], in_=g1q0[:], func=AF.Copy)
            g1e = small.tile([128, 128], F32, tag="g1e", name=f"g1e{si}")
            nc.gpsimd.tensor_sub(g1e[:], g1sp[:], g1dec[:])
            nc.vector.copy_predicated(g1sp[:], m1t[:].to_broadcast([128, 128]),
                                      g1e[:])
            g1q = small.tile([128, 128], F32R, tag="g1q", name=f"g1q{si}")
            nc.scalar.activation(out=g1q[:], in_=g1sp[:], func=AF.Copy)

            # 4 packed n-tiles: K=12 split-fp32r distance matmuls + drains
            dsbs = []
            for g in range(4):
                dsb = dsb_pool.tile([128, S], F32, tag="dsb", name=f"dsb{si}_{g}")
                dsbs.append(dsb)
            for sc in range(4):
                for g in range(4):
                    dp = dpsum.tile([128, 512], F32, tag="dp", name=f"dp{si}_{sc}_{g}")
                    nc.tensor.matmul(
                        out=dp[:],
                        lhsT=g1q[32 * g:32 * g + 12, :],
                        rhs=g2q[32 * g:32 * g + 12, sc * 512:(sc + 1) * 512],
                        start=True, stop=True, tile_position=(32 * g, 0),
                    )
                    dst = dsbs[g][:, sc * 512:(sc + 1) * 512]
                    if sc == 3:
                        nc.vector.tensor_copy(dst, dp[:])
                    else:
                        nc.scalar.activation(out=dst, in_=dp[:], func=AF.Copy)

            xk1 = xkpool.tile([128, 512], F32R, tag="xk1", name=f"xk1_{si}")
            xk2 = xkpool.tile([128, 512], F32R, tag="xk2", name=f"xk2_{si}")

            for g in range(4):
                t = 4 * si + g
                dsb = dsbs[g]
                x1sqt = small.tile([128, 1], F32, tag="x1sqt", name=f"x1sq{t}")
                nc.sync.dma_start(x1sqt[:], x1sq_in[t])

                m8 = small.tile([128, 8], F32, tag="m8", name=f"m8_{t}")
                nc.vector.max(out=m8[:], in_=dsb[:])
                idx8 = small.tile([128, 8], U32, tag="idx8", name=f"idx8_{t}")
                nc.vector.max_index(out=idx8[:], in_max=m8[:], in_values=dsb[:])

                d3 = small.tile([128, 3], F32, tag="d3", name=f"d3_{t}")
                nc.gpsimd.tensor_tensor(out=d3[:], in0=x1sqt[:].to_broadcast([128, 3]),
                                        in1=m8[:, 0:3], op=OP.subtract)
                nc.gpsimd.tensor_scalar_add(d3[:], d3[:], REC_EPS)
                r3 = small.tile([128, 3], F32, tag="r3", name=f"r3_{t}")
                nc.vector.reciprocal(out=r3[:], in_=d3[:])
                rs = small.tile([128, 1], F32, tag="rs", name=f"rs_{t}")
                nc.gpsimd.tensor_add(rs[:], r3[:, 0:1], r3[:, 1:2])
                nc.gpsimd.tensor_add(rs[:], rs[:], r3[:, 2:3])
                rsi_t = small.tile([128, 1], F32, tag="rsi", name=f"rsi_{t}")
                nc.vector.reciprocal(out=rsi_t[:], in_=rs[:])
                w3 = small.tile([128, 3], F32, tag="w3", name=f"w3_{t}")
                nc.gpsimd.tensor_tensor(out=w3[:], in0=r3[:],
                                        in1=rsi_t[:].to_broadcast([128, 3]),
                                        op=OP.mult)

                gj, dj = [], []
                for j in range(3):
                    gt = gpool.tile([128, D2], F32, tag=f"g{j}", name=f"g{t}_{j}")
                    nc.gpsimd.indirect_dma_start(
                        out=gt[:], out_offset=None, in_=p2t_in,
                        in_offset=bass.IndirectOffsetOnAxis(ap=idx8[:, j:j + 1],
                                                            axis=0),
                    )
                    gj.append(gt)
                for j in range(3):
                    d = small.tile([128, 128], F32, tag=f"diag{j}", name=f"dg{t}_{j}")
                    if j == 0:
                        nc.vector.tensor_tensor(
                            out=d[:], in0=ident[:],
                            in1=w3[:, j:j + 1].to_broadcast([128, 128]), op=OP.mult)
                    elif j == 1:
                        nc.scalar.activation(out=d[:], in_=ident[:], func=AF.Copy,
                                             scale=w3[:, j:j + 1])
                    else:
                        nc.gpsimd.tensor_tensor(
                            out=d[:], in0=ident[:],
                            in1=w3[:, j:j + 1].to_broadcast([128, 128]), op=OP.mult)
                    dj.append(d)

                for mo in range(2):
                    it = ipsum.tile([128, 128], F32, tag="it", name=f"it{t}_{mo}")
                    for j in range(3):
                        nc.tensor.matmul(
                            out=it[:], lhsT=gj[j][:, mo * 128:(mo + 1) * 128],
                            rhs=dj[j][:], start=(j == 0), stop=(j == 2),
                        )
                    dst = xk1 if mo == 0 else xk2
                    nc.scalar.activation(out=dst[:, g * 128:(g + 1) * 128], in_=it[:],
                                         func=AF.Copy)

            # MLP layer 1 on chunk ci = si (fp32r)
            ci = si
            p1c = p1pool.tile([128, 512], F32, tag="p1c", name=f"p1c{ci}")
            nc.sync.dma_start(p1c[:], p1_in[:, ci * 512:(ci + 1) * 512])
            p1r = p1pool.tile([128, 512], F32R, tag="p1r", name=f"p1r{ci}")
            nc.scalar.activation(out=p1r[:], in_=p1c[:], func=AF.Copy)
            for mo in range(2):
                yp = mpsum.tile([128, 512], F32, tag="yp", name=f"yp{ci}_{mo}")
                nc.tensor.matmul(out=yp[:], lhsT=w0t[:, 0, mo, :], rhs=p1r[:],
                                 start=True, stop=False)
                nc.tensor.matmul(out=yp[:], lhsT=w0t[:, 1, mo, :], rhs=xk1[:],
                                 start=False, stop=False)
                nc.tensor.matmul(out=yp[:], lhsT=w0t[:, 2, mo, :], rhs=xk2[:],
                                 start=False, stop=True)
                ysl = y1sb[mo][:, ci * 512:(ci + 1) * 512]
                nc.scalar.activation(out=ysl, in_=yp[:], func=AF.Copy,
                                     accum_out=s1acc[:, mo, ci:ci + 1])
                dump = dumpp.tile([128, 512], F32, tag="dump", name=f"du{ci}_{mo}")
                nc.scalar.activation(out=dump[:], in_=ysl, func=AF.Square,
                                     accum_out=s2acc[:, mo, ci:ci + 1])

        # ---------- BN1 ----------
        scale0, shift0 = _bn_allreduce(tc, nc, singles, small, dram, epst, affs,
                                       s1acc, s2acc, layer=0)

        # ---------- phase 2: normalize+relu y1 (fp32r), MLP layer 2 ----------
        for ci in range(NCH):
            y1n = []
            for mo in range(2):
                yn = ypool.tile([128, 512], F32R, tag=f"y1n{mo}", name=f"y1n{ci}_{mo}")
                src = y1sb[mo][:, ci * 512:(ci + 1) * 512]
                if mo == 0:
                    nc.scalar.activation(out=yn[:], in_=src, func=AF.Relu,
                                         bias=shift0[mo][:], scale=scale0[mo][:])
                else:
                    nc.vector.tensor_scalar(out=yn[:], in0=src,
                                            scalar1=scale0[mo][:],
                                            scalar2=shift0[mo][:],
                                            op0=OP.mult, op1=OP.add)
                    nc.vector.tensor_scalar_max(yn[:], yn[:], 0.0)
                y1n.append(yn)
            for mo in range(2):
                yp = mpsum.tile([128, 512], F32, tag="yp", name=f"y2p{ci}_{mo}")
                nc.tensor.matmul(out=yp[:], lhsT=w1t[:, 0, mo, :], rhs=y1n[0][:],
                                 start=True, stop=False)
                nc.tensor.matmul(out=yp[:], lhsT=w1t[:, 1, mo, :], rhs=y1n[1][:],
                                 start=False, stop=True)
                y2c = ypool.tile([128, 512], F32, tag="y2c", name=f"y2c{ci}_{mo}")
                nc.scalar.activation(out=y2c[:], in_=yp[:], func=AF.Copy,
                                     accum_out=t1acc[:, mo, ci:ci + 1])
                dump = dumpp.tile([128, 512], F32, tag="dump2", name=f"d2{ci}_{mo}")
                nc.scalar.activation(out=dump[:], in_=y2c[:], func=AF.Square,
                                     accum_out=t2acc[:, mo, ci:ci + 1])
                nc.sync.dma_start(y2_dram[mo, :, ci * 512:(ci + 1) * 512], y2c[:])

        # ---------- BN2 ----------
        scale1, shift1 = _bn_allreduce(tc, nc, singles, small, dram, epst, affs,
                                       t1acc, t2acc, layer=1)

        # ---------- phase 3 ----------
        for ci in range(NCH):
            for mo in range(2):
                y2c = ypool.tile([128, 512], F32, tag="y2r", name=f"y2r{ci}_{mo}")
                nc.sync.dma_start(y2c[:], y2_dram[mo, :, ci * 512:(ci + 1) * 512])
                oc = ypool.tile([128, 512], F32, tag="oc", name=f"oc{ci}_{mo}")
                if mo == 0:
                    nc.scalar.activation(out=oc[:], in_=y2c[:], func=AF.Relu,
                                         bias=shift1[mo][:], scale=scale1[mo][:])
                else:
                    nc.vector.tensor_scalar(out=oc[:], in0=y2c[:],
                                            scalar1=scale1[mo][:],
                                            scalar2=shift1[mo][:],
                                            op0=OP.mult, op1=OP.add)
                    nc.vector.tensor_scalar_max(oc[:], oc[:], 0.0)
                nc.sync.dma_start(
                    out_ext[mo * 128:(mo + 1) * 128, ci * 512:(ci + 1) * 512], oc[:])


def _bn_allreduce(tc, nc, singles, small, dram, epst, affs, sacc, sqacc, layer):
    """Reduce per-chunk partials, AllReduce over 8 cores, return per-chunk
    (scale, shift) [128,1] tiles so BN+affine is y*scale + shift."""
    stats = singles.tile([128, 4], F32, tag=f"stats{layer}", name=f"stats{layer}")
    for mo in range(2):
        sdump = small.tile([128, NCH], F32, tag="sdump", name=f"sdump{layer}{mo}")
        nc.scalar.activation(out=sdump[:], in_=sacc[:, mo, :], func=AF.Copy,
                             accum_out=stats[:, mo:mo + 1])
        qdump = small.tile([128, NCH], F32, tag="qdump", name=f"qdump{layer}{mo}")
        nc.scalar.activation(out=qdump[:], in_=sqacc[:, mo, :], func=AF.Copy,
                             accum_out=stats[:, 2 + mo:3 + mo])
    inb = dram.tile([128, 4], F32, tag=f"arin{layer}", name=f"arin{layer}")
    outb = dram.tile([128, 4], F32, tag=f"arout{layer}", name=f"arout{layer}")
    nc.gpsimd.dma_start(inb[:], stats[:])
    nc.gpsimd.collective_compute(
        "AllReduce", mybir.AluOpType.add,
        replica_groups=[list(range(8))],
        ins=[inb.opt()], outs=[outb.opt()],
    )
    gstats = singles.tile([128, 4], F32, tag=f"gstats{layer}", name=f"gstats{layer}")
    nc.gpsimd.dma_start(gstats[:], outb[:])

    scale, shift = [], []
    for mo in range(2):
        mean = singles.tile([128, 1], F32, tag=f"mean{layer}{mo}",
                            name=f"mean{layer}{mo}")
        nc.gpsimd.tensor_scalar_mul(mean[:], gstats[:, mo:mo + 1], 1.0 / CNT)
        var = singles.tile([128, 1], F32, tag=f"var{layer}{mo}",
                           name=f"var{layer}{mo}")
        nc.gpsimd.tensor_scalar_mul(var[:], gstats[:, 2 + mo:3 + mo], 1.0 / CNT)
        msq = small.tile([128, 1], F32, tag="msq", name=f"msq{layer}{mo}")
        nc.gpsimd.tensor_mul(msq[:], mean[:], mean[:])
        nc.gpsimd.tensor_sub(var[:], var[:], msq[:])
        sd = singles.tile([128, 1], F32, tag=f"sd_{layer}{mo}", name=f"sd{layer}{mo}")
        nc.scalar.activation(out=sd[:], in_=var[:], func=AF.Sqrt, bias=epst[:])
        rsd = singles.tile([128, 1], F32, tag=f"rsd{layer}{mo}",
                           name=f"rsd{layer}{mo}")
        nc.vector.reciprocal(out=rsd[:], in_=sd[:])
        sc = singles.tile([128, 1], F32, tag=f"scale{layer}{mo}",
                          name=f"scale{layer}{mo}")
        nc.gpsimd.tensor_mul(sc[:], affs[:, layer, mo, 0:1], rsd[:])
        sh = singles.tile([128, 1], F32, tag=f"shift{layer}{mo}",
                          name=f"shift{layer}{mo}")
        nc.gpsimd.tensor_mul(sh[:], mean[:], sc[:])
        nc.gpsimd.tensor_sub(sh[:], affs[:, layer, mo, 1:2], sh[:])
        scale.append(sc)
        shift.append(sh)
    return scale, shift


def _prep_core_inputs(b, xyz1, xyz2, points1, points2, W0, W1, g0, beta0, g1, beta1):
    x1 = xyz1[b].astype(np.float32)          # [3, N]
    x2 = xyz2[b].astype(np.float32)          # [3, S]
    x1sq = (x1 * x1).sum(0).astype(np.float32)
    x2sq = (x2 * x2).sum(0).astype(np.float32)
    g1m = np.concatenate([x1, np.ones((1, N), np.float32)], 0)   # [4, N]
    g1t = g1m.reshape(4, NT, 128).transpose(1, 0, 2)             # [NT, 4, 128]
    # packed split-fp32r layout: super-tile si holds tile 4si+g at partition
    # offset 32g, tripled into rows +0..3 (-> r), +4..7 (-> e), +8..11 (-> r)
    g1p = np.zeros((NSI, 128, 128), np.float32)
    for g in range(4):
        for blk in range(3):
            g1p[:, 32 * g + 4 * blk:32 * g + 4 * blk + 4, :] = g1t[g::4][:NSI]
    g2m = np.concatenate([2.0 * x2, -x2sq[None]], 0).astype(np.float32)  # [4, S]
    g2rep = np.zeros((128, S), np.float32)
    for g in range(4):
        for blk in range(3):
            g2rep[32 * g + 4 * blk:32 * g + 4 * blk + 4, :] = g2m
    pidx = np.arange(128) % 32
    m1 = ((pidx >= 4) & (pidx < 8)).astype(np.uint32).reshape(128, 1)
    m2 = ((pidx >= 8) & (pidx < 12)).astype(np.uint32).reshape(128, 1)
    w0t = np.ascontiguousarray(
        W0.T.reshape(3, 128, 2, 128).transpose(1, 0, 2, 3)).astype(np.float32)
    w1t = np.ascontiguousarray(
        W1.T.reshape(2, 128, 2, 128).transpose(1, 0, 2, 3)).astype(np.float32)
    aff = np.stack([
        np.stack([np.stack([g0.reshape(2, 128)[c], beta0.reshape(2, 128)[c]], -1)
                  for c in range(2)]),
        np.stack([np.stack([g1.reshape(2, 128)[c], beta1.reshape(2, 128)[c]], -1)
                  for c in range(2)]),
    ]).astype(np.float32)
    return {
        "g1p": g1p,
        "g2r": g2rep,
        "x1sq": np.ascontiguousarray(x1sq.reshape(NT, 128, 1)),
        "p2t": np.ascontiguousarray(points2[b].T).astype(np.float32),
        "p1": np.ascontiguousarray(points1[b]).astype(np.float32),
        "w0t": w0t,
        "w1t": w1t,
        "aff": aff,
        "m1": m1,
        "m2": m2,
    }


def kernel(xyz1, xyz2, points1, points2, W0, b0, g0, beta0, W1, b1, g1, beta1,
           **_ignored):
    # b0/b1 cancel exactly in training-mode BatchNorm (constant channel shift
    # moves y and its mean equally) so they are not used on device.
    if "nc" not in _CACHE:
        _CACHE["nc"] = _build_program()
    nc = _CACHE["nc"]
    in_maps = [
        _prep_core_inputs(b, np.asarray(xyz1), np.asarray(xyz2),
                          np.asarray(points1), np.asarray(points2),
                          np.asarray(W0), np.asarray(W1),
                          np.asarray(g0), np.asarray(beta0),
                          np.asarray(g1), np.asarray(beta1))
        for b in range(B)
    ]
    res = run_bass_kernel_spmd(nc, in_maps, list(range(8)))
    out = np.stack([res.results[c]["out"] for c in range(8)], axis=0)
    return out.astype(np.float32)


# revision 33
# speedup vs baseline: 1.2628x; 1.0184x over previous
# PointNet Feature Propagation kernel for Trainium2 (8 NeuronCores, SPMD).
#
# Sharding: data-parallel over batch B=8 -> 1 batch element per core.
# Per core:
#   1. negd'[n,s] = 2*x1.x2 - |x2|^2 via K=4 matmuls, 4 n-tiles packed into
#      the PE array concurrently with tile_position row groups (fp32).
#   2. nc.vector.max (top-8 values) + max_index (indices; HW returns
#      ascending distinct indices for exact ties, matching jax top_k).
#   3. Weights w_j from recovered distances d_j = |x1|^2 - m_j.
#   4. Gather p2^T rows via gpsimd indirect DMA (3 x [128,1]-offset calls).
#   5. interp^T = sum_j g_j^T @ diag(w_j) on PE (weighted-sum + transpose).
#   6. 2-layer 1x1-conv MLP on PE in fp32r; BatchNorm stats via ACT
#      accum_out with cross-core AllReduce; normalize+ReLU fused in ACT.
import numpy as np

import concourse.bass as bass
import concourse.bacc as bacc
import concourse.tile as tile
from concourse import mybir
from concourse.bass_utils import run_bass_kernel_spmd
from concourse.masks import make_identity

B, N, S = 8, 8192, 2048
D1, D2 = 128, 256
BN_EPS = 1e-5
REC_EPS = 1e-8
NT = N // 128          # 64 n-tiles
NSI = NT // 4          # 16 super-iterations (4 packed n-tiles each = 512 n)
NCH = N // 512         # 16 MLP chunks
CNT = float(B * N)     # BN population
F32 = mybir.dt.float32
F32R = mybir.dt.float32r
U32 = mybir.dt.uint32
AF = mybir.ActivationFunctionType
OP = mybir.AluOpType

_CACHE = {}


def _build_program():
    nc = bacc.Bacc("TRN2", target_bir_lowering=False, debug=False, num_devices=8)

    g1p_in = nc.dram_tensor("g1p", [NSI, 128, 128], F32, kind="ExternalInput").ap()
    g2r_in = nc.dram_tensor("g2r", [128, S], F32, kind="ExternalInput").ap()
    x1sq_in = nc.dram_tensor("x1sq", [NT, 128, 1], F32, kind="ExternalInput").ap()
    p2t_in = nc.dram_tensor("p2t", [S, D2], F32, kind="ExternalInput").ap()
    p1_in = nc.dram_tensor("p1", [D1, N], F32, kind="ExternalInput").ap()
    w0t_in = nc.dram_tensor("w0t", [128, 3, 2, 128], F32, kind="ExternalInput").ap()
    w1t_in = nc.dram_tensor("w1t", [128, 2, 2, 128], F32, kind="ExternalInput").ap()
    aff_in = nc.dram_tensor("aff", [2, 2, 128, 2], F32, kind="ExternalInput").ap()
    m1_in = nc.dram_tensor("m1", [128, 1], U32, kind="ExternalInput").ap()
    m2_in = nc.dram_tensor("m2", [128, 1], U32, kind="ExternalInput").ap()
    out_ext = nc.dram_tensor("out", [D2, N], F32, kind="ExternalOutput").ap()
    y2_dram = nc.dram_tensor("y2stage", [2, 128, N], F32).ap()

    with tile.TileContext(nc) as tc:
        _emit(tc, nc, g1p_in, g2r_in, x1sq_in, p2t_in, p1_in, w0t_in, w1t_in,
              aff_in, out_ext, y2_dram, m1_in, m2_in)
    nc.compile()
    return nc


def _emit(tc, nc, g1p_in, g2r_in, x1sq_in, p2t_in, p1_in, w0t_in, w1t_in,
          aff_in, out_ext, y2_dram, m1_in, m2_in):
    from contextlib import ExitStack
    ctx = ExitStack()
    with ctx:
        singles = ctx.enter_context(tc.tile_pool(name="singles", bufs=1))
        dpsum = ctx.enter_context(tc.tile_pool(name="dpsum", bufs=4, space="PSUM"))
        ipsum = ctx.enter_context(tc.tile_pool(name="ipsum", bufs=2, space="PSUM"))
        mpsum = ctx.enter_context(tc.tile_pool(name="mpsum", bufs=2, space="PSUM"))
        dsb_pool = ctx.enter_context(tc.tile_pool(name="dsb", bufs=4))
        small = ctx.enter_context(tc.tile_pool(name="small", bufs=4))
        gpool = ctx.enter_context(tc.tile_pool(name="gath", bufs=3))
        xkpool = ctx.enter_context(tc.tile_pool(name="xk", bufs=2))
        p1pool = ctx.enter_context(tc.tile_pool(name="p1c", bufs=2))
        ypool = ctx.enter_context(tc.tile_pool(name="ych", bufs=2))
        dumpp = ctx.enter_context(tc.tile_pool(name="dump", bufs=2))
        dram = ctx.enter_context(tc.tile_pool(name="dramb", bufs=1, space="DRAM"))

        # ---------- resident setup ----------
        m1t = singles.tile([128, 1], U32)
        nc.sync.dma_start(m1t[:], m1_in)
        m2t = singles.tile([128, 1], U32)
        nc.sync.dma_start(m2t[:], m2_in)
        # G2 split-fp32r prep: rows 32g+0..3 r2, +4..7 r2(dup), +8..11 e2.
        # Temporaries borrow the dsb pool's rotating [128, S] slots.
        g2raw = dsb_pool.tile([128, S], F32, tag="dsb", name="g2raw")
        nc.sync.dma_start(g2raw[:], g2r_in)
        g2q = singles.tile([128, S], F32R)
        nc.vector.tensor_copy(g2q[:], g2raw[:])
        g2dec = dsb_pool.tile([128, S], F32, tag="dsb", name="g2dec")
        nc.vector.tensor_copy(g2dec[:], g2q[:])
        g2e = dsb_pool.tile([128, S], F32, tag="dsb", name="g2e")
        nc.vector.tensor_sub(g2e[:], g2raw[:], g2dec[:])
        nc.vector.copy_predicated(g2raw[:], m2t[:].to_broadcast([128, S]), g2e[:])
        nc.vector.tensor_copy(g2q[:], g2raw[:])
        ident = singles.tile([128, 128], F32)
        make_identity(nc, ident[:])
        w0f = singles.tile([128, 3, 2, 128], F32)
        nc.sync.dma_start(w0f[:], w0t_in)
        w1f = singles.tile([128, 2, 2, 128], F32)
        nc.sync.dma_start(w1f[:], w1t_in)
        # fp32r-rounded copies for the MLP matmuls
        w0t = singles.tile([128, 3, 2, 128], F32R)
        nc.vector.tensor_copy(w0t[:], w0f[:])
        w1t = singles.tile([128, 2, 2, 128], F32R)
        nc.vector.tensor_copy(w1t[:], w1f[:])
        affs = singles.tile([128, 2, 2, 2], F32)  # [part, layer, chunk, g/b]
        for l in range(2):
            for mo in range(2):
                nc.sync.dma_start(affs[:, l, mo, :], aff_in[l, mo])
        epst = singles.tile([128, 1], F32)
        nc.vector.memset(epst[:], BN_EPS)
        y1sb = [singles.tile([128, N], F32, tag=f"y1sb{mo}", name=f"y1sb{mo}")
                for mo in range(2)]
        s1acc = singles.tile([128, 2, NCH], F32)
        s2acc = singles.tile([128, 2, NCH], F32)
        t1acc = singles.tile([128, 2, NCH], F32)
        t2acc = singles.tile([128, 2, NCH], F32)

        # ---------- phase 1 ----------
        for si in range(NSI):
            # G1 split-fp32r prep per super-tile
            g1sp = small.tile([128, 128], F32, tag="g1sp", name=f"g1sp{si}")
            nc.sync.dma_start(g1sp[:], g1p_in[si])
            g1q0 = small.tile([128, 128], F32R, tag="g1q0", name=f"g1q0{si}")
            nc.scalar.activation(out=g1q0[:], in_=g1sp[:], func=AF.Copy)
            g1dec = small.tile([128, 128], F32, tag="g1dec", name=f"g1dec{si}")
            nc.scalar.activation(out=g1dec[:], in_=g1q0[:], func=AF.Copy)
            g1e = small.tile([128, 128], F32, tag="g1e", name=f"g1e{si}")
            nc.gpsimd.tensor_sub(g1e[:], g1sp[:], g1dec[:])
            nc.vector.copy_predicated(g1sp[:], m1t[:].to_broadcast([128, 128]),
                                      g1e[:])
            g1q = small.tile([128, 128], F32R, tag="g1q", name=f"g1q{si}")
            nc.scalar.activation(out=g1q[:], in_=g1sp[:], func=AF.Copy)

            # 4 packed n-tiles: K=12 split-fp32r distance matmuls + drains
            dsbs = []
            for g in range(4):
                dsb = dsb_pool.tile([128, S], F32, tag="dsb", name=f"dsb{si}_{g}")
                dsbs.append(dsb)
            for sc in range(4):
                for g in range(4):
                    dp = dpsum.tile([128, 512], F32, tag="dp", name=f"dp{si}_{sc}_{g}")
                    nc.tensor.matmul(
                        out=dp[:],
                        lhsT=g1q[32 * g:32 * g + 12, :],
                        rhs=g2q[32 * g:32 * g + 12, sc * 512:(sc + 1) * 512],
                        start=True, stop=True, tile_position=(32 * g, 0),
                    )
                    dst = dsbs[g][:, sc * 512:(sc + 1) * 512]
                    if sc == 3:
                        nc.vector.tensor_copy(dst, dp[:])
                    else:
                        nc.scalar.activation(out=dst, in_=dp[:], func=AF.Copy)

            xk12 = xkpool.tile([128, 1024], F32R, tag="xk12", name=f"xk12_{si}")

            for g in range(4):
                t = 4 * si + g
                dsb = dsbs[g]
                x1sqt = small.tile([128, 1], F32, tag="x1sqt", name=f"x1sq{t}")
                nc.sync.dma_start(x1sqt[:], x1sq_in[t])

                m8 = small.tile([128, 8], F32, tag="m8", name=f"m8_{t}")
                nc.vector.max(out=m8[:], in_=dsb[:])
                idx8 = small.tile([128, 8], U32, tag="idx8", name=f"idx8_{t}")
                nc.vector.max_index(out=idx8[:], in_max=m8[:], in_values=dsb[:])

                # d_j = (x1sq + eps) - m_j   (eps pre-added on host)
                d3 = small.tile([128, 3], F32, tag="d3", name=f"d3_{t}")
                nc.scalar.activation(out=d3[:], in_=m8[:, 0:3], func=AF.Identity,
                                     scale=-1.0, bias=x1sqt[:])
                r3 = small.tile([128, 3], F32, tag="r3", name=f"r3_{t}")
                nc.vector.reciprocal(out=r3[:], in_=d3[:])
                rs = small.tile([128, 1], F32, tag="rs", name=f"rs_{t}")
                rdump = small.tile([128, 3], F32, tag="rdump", name=f"rdump{t}")
                nc.scalar.activation(out=rdump[:], in_=r3[:], func=AF.Copy,
                                     accum_out=rs[:])
                rsi_t = small.tile([128, 1], F32, tag="rsi", name=f"rsi_{t}")
                nc.vector.reciprocal(out=rsi_t[:], in_=rs[:])
                w3 = small.tile([128, 3], F32, tag="w3", name=f"w3_{t}")
                nc.gpsimd.tensor_tensor(out=w3[:], in0=r3[:],
                                        in1=rsi_t[:].to_broadcast([128, 3]),
                                        op=OP.mult)

                gj, dj = [], []
                for j in range(3):
                    gt = gpool.tile([128, D2], F32, tag=f"g{j}", name=f"g{t}_{j}")
                    nc.gpsimd.indirect_dma_start(
                        out=gt[:], out_offset=None, in_=p2t_in,
                        in_offset=bass.IndirectOffsetOnAxis(ap=idx8[:, j:j + 1],
                                                            axis=0),
                    )
                    gj.append(gt)
                for j in range(3):
                    d = small.tile([128, 128], F32, tag=f"diag{j}", name=f"dg{t}_{j}")
                    if j == 0:
                        nc.vector.tensor_tensor(
                            out=d[:], in0=ident[:],
                            in1=w3[:, j:j + 1].to_broadcast([128, 128]), op=OP.mult)
                    elif j == 1:
                        nc.scalar.activation(out=d[:], in_=ident[:], func=AF.Copy,
                                             scale=w3[:, j:j + 1])
                    else:
                        nc.gpsimd.tensor_tensor(
                            out=d[:], in0=ident[:],
                            in1=w3[:, j:j + 1].to_broadcast([128, 128]), op=OP.mult)
                    dj.append(d)

                it = ipsum.tile([128, 256], F32, tag="it", name=f"it{t}")
                for mo in range(2):
                    for j in range(3):
                        nc.tensor.matmul(
                            out=it[:, mo * 128:(mo + 1) * 128],
                            lhsT=gj[j][:, mo * 128:(mo + 1) * 128],
                            rhs=dj[j][:], start=(j == 0), stop=(j == 2),
                        )
                xkv = xk12[:].rearrange("p (k n) -> p k n", k=2)
                nc.scalar.activation(out=xkv[:, :, g * 128:(g + 1) * 128],
                                     in_=it[:].rearrange("p (k n) -> p k n", k=2),
                                     func=AF.Copy)

            # MLP layer 1 on chunk ci = si (fp32r)
            ci = si
            p1c = p1pool.tile([128, 512], F32, tag="p1c", name=f"p1c{ci}")
            nc.sync.dma_start(p1c[:], p1_in[:, ci * 512:(ci + 1) * 512])
            p1r = p1pool.tile([128, 512], F32R, tag="p1r", name=f"p1r{ci}")
            nc.scalar.activation(out=p1r[:], in_=p1c[:], func=AF.Copy)
            for mo in range(2):
                yp = mpsum.tile([128, 512], F32, tag="yp", name=f"yp{ci}_{mo}")
                nc.tensor.matmul(out=yp[:], lhsT=w0t[:, 0, mo, :], rhs=p1r[:],
                                 start=True, stop=False)
                nc.tensor.matmul(out=yp[:], lhsT=w0t[:, 1, mo, :],
                                 rhs=xk12[:, 0:512], start=False, stop=False)
                nc.tensor.matmul(out=yp[:], lhsT=w0t[:, 2, mo, :],
                                 rhs=xk12[:, 512:1024], start=False, stop=True)
                ysl = y1sb[mo][:, ci * 512:(ci + 1) * 512]
                nc.scalar.activation(out=ysl, in_=yp[:], func=AF.Copy,
                                     accum_out=s1acc[:, mo, ci:ci + 1])
                dump = dumpp.tile([128, 512], F32, tag="dump", name=f"du{ci}_{mo}")
                nc.scalar.activation(out=dump[:], in_=ysl, func=AF.Square,
                                     accum_out=s2acc[:, mo, ci:ci + 1])

        # ---------- BN1 ----------
        scale0, shift0 = _bn_allreduce(tc, nc, singles, small, dram, epst, affs,
                                       s1acc, s2acc, layer=0)

        # ---------- phase 2: normalize+relu y1 (fp32r), MLP layer 2 ----------
        for ci in range(NCH):
            y1n = []
            for mo in range(2):
                yn = ypool.tile([128, 512], F32R, tag=f"y1n{mo}", name=f"y1n{ci}_{mo}")
                src = y1sb[mo][:, ci * 512:(ci + 1) * 512]
                if mo == 0:
                    nc.scalar.activation(out=yn[:], in_=src, func=AF.Relu,
                                         bias=shift0[mo][:], scale=scale0[mo][:])
                else:
                    nc.vector.tensor_scalar(out=yn[:], in0=src,
                                            scalar1=scale0[mo][:],
                                            scalar2=shift0[mo][:],
                                            op0=OP.mult, op1=OP.add)
                    nc.vector.tensor_scalar_max(yn[:], yn[:], 0.0)
                y1n.append(yn)
            for mo in range(2):
                yp = mpsum.tile([128, 512], F32, tag="yp", name=f"y2p{ci}_{mo}")
                nc.tensor.matmul(out=yp[:], lhsT=w1t[:, 0, mo, :], rhs=y1n[0][:],
                                 start=True, stop=False)
                nc.tensor.matmul(out=yp[:], lhsT=w1t[:, 1, mo, :], rhs=y1n[1][:],
                                 start=False, stop=True)
                y2c = ypool.tile([128, 512], F32, tag="y2c", name=f"y2c{ci}_{mo}")
                if mo == 0:
                    nc.scalar.activation(out=y2c[:], in_=yp[:], func=AF.Copy,
                                         accum_out=t1acc[:, mo, ci:ci + 1])
                else:
                    nc.vector.tensor_scalar(out=y2c[:], in0=yp[:], scalar1=1.0,
                                            scalar2=0.0, op0=OP.mult, op1=OP.add,
                                            accum_out=t1acc[:, mo, ci:ci + 1])
                dump = dumpp.tile([128, 512], F32, tag="dump2", name=f"d2{ci}_{mo}")
                nc.vector.scalar_tensor_tensor(out=dump[:], in0=y2c[:], scalar=1.0,
                                               in1=y2c[:], op0=OP.mult, op1=OP.mult,
                                               accum_out=t2acc[:, mo, ci:ci + 1])
                nc.sync.dma_start(y2_dram[mo, :, ci * 512:(ci + 1) * 512], y2c[:])

        # ---------- BN2 ----------
        scale1, shift1 = _bn_allreduce(tc, nc, singles, small, dram, epst, affs,
                                       t1acc, t2acc, layer=1)

        # ---------- phase 3 ----------
        for ci in range(NCH):
            for mo in range(2):
                y2c = ypool.tile([128, 512], F32, tag="y2r", name=f"y2r{ci}_{mo}")
                nc.scalar.dma_start(y2c[:], y2_dram[mo, :, ci * 512:(ci + 1) * 512])
                oc = ypool.tile([128, 512], F32, tag="oc", name=f"oc{ci}_{mo}")
                if mo == 0:
                    nc.scalar.activation(out=oc[:], in_=y2c[:], func=AF.Relu,
                                         bias=shift1[mo][:], scale=scale1[mo][:])
                else:
                    nc.vector.tensor_scalar(out=oc[:], in0=y2c[:],
                                            scalar1=scale1[mo][:],
                                            scalar2=shift1[mo][:],
                                            op0=OP.mult, op1=OP.add)
                    nc.vector.tensor_scalar_max(oc[:], oc[:], 0.0)
                nc.sync.dma_start(
                    out_ext[mo * 128:(mo + 1) * 128, ci * 512:(ci + 1) * 512], oc[:])


def _bn_allreduce(tc, nc, singles, small, dram, epst, affs, sacc, sqacc, layer):
    """Reduce per-chunk partials, AllReduce over 8 cores, return per-chunk
    (scale, shift) [128,1] tiles so BN+affine is y*scale + shift."""
    stats = singles.tile([128, 4], F32, tag=f"stats{layer}", name=f"stats{layer}")
    for mo in range(2):
        sdump = small.tile([128, NCH], F32, tag="sdump", name=f"sdump{layer}{mo}")
        nc.scalar.activation(out=sdump[:], in_=sacc[:, mo, :], func=AF.Copy,
                             accum_out=stats[:, mo:mo + 1])
        qdump = small.tile([128, NCH], F32, tag="qdump", name=f"qdump{layer}{mo}")
        nc.scalar.activation(out=qdump[:], in_=sqacc[:, mo, :], func=AF.Copy,
                             accum_out=stats[:, 2 + mo:3 + mo])
    inb = dram.tile([128, 4], F32, tag=f"arin{layer}", name=f"arin{layer}")
    outb = dram.tile([128, 4], F32, tag=f"arout{layer}", name=f"arout{layer}")
    nc.gpsimd.dma_start(inb[:], stats[:])
    nc.gpsimd.collective_compute(
        "AllReduce", mybir.AluOpType.add,
        replica_groups=[list(range(8))],
        ins=[inb.opt()], outs=[outb.opt()],
    )
    gstats = singles.tile([128, 4], F32, tag=f"gstats{layer}", name=f"gstats{layer}")
    nc.gpsimd.dma_start(gstats[:], outb[:])

    scale, shift = [], []
    for mo in range(2):
        mean = singles.tile([128, 1], F32, tag=f"mean{layer}{mo}",
                            name=f"mean{layer}{mo}")
        nc.gpsimd.tensor_scalar_mul(mean[:], gstats[:, mo:mo + 1], 1.0 / CNT)
        var = singles.tile([128, 1], F32, tag=f"var{layer}{mo}",
                           name=f"var{layer}{mo}")
        nc.gpsimd.tensor_scalar_mul(var[:], gstats[:, 2 + mo:3 + mo], 1.0 / CNT)
        msq = small.tile([128, 1], F32, tag="msq", name=f"msq{layer}{mo}")
        nc.gpsimd.tensor_mul(msq[:], mean[:], mean[:])
        nc.gpsimd.tensor_sub(var[:], var[:], msq[:])
        sd = singles.tile([128, 1], F32, tag=f"sd_{layer}{mo}", name=f"sd{layer}{mo}")
        nc.scalar.activation(out=sd[:], in_=var[:], func=AF.Sqrt, bias=epst[:])
        rsd = singles.tile([128, 1], F32, tag=f"rsd{layer}{mo}",
                           name=f"rsd{layer}{mo}")
        nc.vector.reciprocal(out=rsd[:], in_=sd[:])
        sc = singles.tile([128, 1], F32, tag=f"scale{layer}{mo}",
                          name=f"scale{layer}{mo}")
        nc.gpsimd.tensor_mul(sc[:], affs[:, layer, mo, 0:1], rsd[:])
        sh = singles.tile([128, 1], F32, tag=f"shift{layer}{mo}",
                          name=f"shift{layer}{mo}")
        nc.gpsimd.tensor_mul(sh[:], mean[:], sc[:])
        nc.gpsimd.tensor_sub(sh[:], affs[:, layer, mo, 1:2], sh[:])
        scale.append(sc)
        shift.append(sh)
    return scale, shift


def _prep_core_inputs(b, xyz1, xyz2, points1, points2, W0, W1, g0, beta0, g1, beta1):
    x1 = xyz1[b].astype(np.float32)          # [3, N]
    x2 = xyz2[b].astype(np.float32)          # [3, S]
    x1sq = ((x1 * x1).sum(0) + np.float32(REC_EPS)).astype(np.float32)
    x2sq = (x2 * x2).sum(0).astype(np.float32)
    g1m = np.concatenate([x1, np.ones((1, N), np.float32)], 0)   # [4, N]
    g1t = g1m.reshape(4, NT, 128).transpose(1, 0, 2)             # [NT, 4, 128]
    # packed split-fp32r layout: super-tile si holds tile 4si+g at partition
    # offset 32g, tripled into rows +0..3 (-> r), +4..7 (-> e), +8..11 (-> r)
    g1p = np.zeros((NSI, 128, 128), np.float32)
    for g in range(4):
        for blk in range(3):
            g1p[:, 32 * g + 4 * blk:32 * g + 4 * blk + 4, :] = g1t[g::4][:NSI]
    g2m = np.concatenate([2.0 * x2, -x2sq[None]], 0).astype(np.float32)  # [4, S]
    g2rep = np.zeros((128, S), np.float32)
    for g in range(4):
        for blk in range(3):
            g2rep[32 * g + 4 * blk:32 * g + 4 * blk + 4, :] = g2m
    pidx = np.arange(128) % 32
    m1 = ((pidx >= 4) & (pidx < 8)).astype(np.uint32).reshape(128, 1)
    m2 = ((pidx >= 8) & (pidx < 12)).astype(np.uint32).reshape(128, 1)
    w0t = np.ascontiguousarray(
        W0.T.reshape(3, 128, 2, 128).transpose(1, 0, 2, 3)).astype(np.float32)
    w1t = np.ascontiguousarray(
        W1.T.reshape(2, 128, 2, 128).transpose(1, 0, 2, 3)).astype(np.float32)
    aff = np.stack([
        np.stack([np.stack([g0.reshape(2, 128)[c], beta0.reshape(2, 128)[c]], -1)
                  for c in range(2)]),
        np.stack([np.stack([g1.reshape(2, 128)[c], beta1.reshape(2, 128)[c]], -1)
                  for c in range(2)]),
    ]).astype(np.float32)
    return {
        "g1p": g1p,
        "g2r": g2rep,
        "x1sq": np.ascontiguousarray(x1sq.reshape(NT, 128, 1)),
        "p2t": np.ascontiguousarray(points2[b].T).astype(np.float32),
        "p1": np.ascontiguousarray(points1[b]).astype(np.float32),
        "w0t": w0t,
        "w1t": w1t,
        "aff": aff,
        "m1": m1,
        "m2": m2,
    }


def kernel(xyz1, xyz2, points1, points2, W0, b0, g0, beta0, W1, b1, g1, beta1,
           **_ignored):
    # b0/b1 cancel exactly in training-mode BatchNorm (constant channel shift
    # moves y and its mean equally) so they are not used on device.
    if "nc" not in _CACHE:
        _CACHE["nc"] = _build_program()
    nc = _CACHE["nc"]
    in_maps = [
        _prep_core_inputs(b, np.asarray(xyz1), np.asarray(xyz2),
                          np.asarray(points1), np.asarray(points2),
                          np.asarray(W0), np.asarray(W1),
                          np.asarray(g0), np.asarray(beta0),
                          np.asarray(g1), np.asarray(beta1))
        for b in range(B)
    ]
    res = run_bass_kernel_spmd(nc, in_maps, list(range(8)))
    out = np.stack([res.results[c]["out"] for c in range(8)], axis=0)
    return out.astype(np.float32)


# revision 34
# speedup vs baseline: 1.2674x; 1.0037x over previous
# PointNet Feature Propagation kernel for Trainium2 (8 NeuronCores, SPMD).
#
# Sharding: data-parallel over batch B=8 -> 1 batch element per core.
# Per core:
#   1. negd'[n,s] = 2*x1.x2 - |x2|^2 via K=4 matmuls, 4 n-tiles packed into
#      the PE array concurrently with tile_position row groups (fp32).
#   2. nc.vector.max (top-8 values) + max_index (indices; HW returns
#      ascending distinct indices for exact ties, matching jax top_k).
#   3. Weights w_j from recovered distances d_j = |x1|^2 - m_j.
#   4. Gather p2^T rows via gpsimd indirect DMA (3 x [128,1]-offset calls).
#   5. interp^T = sum_j g_j^T @ diag(w_j) on PE (weighted-sum + transpose).
#   6. 2-layer 1x1-conv MLP on PE in fp32r; BatchNorm stats via ACT
#      accum_out with cross-core AllReduce; normalize+ReLU fused in ACT.
import numpy as np

import concourse.bass as bass
import concourse.bacc as bacc
import concourse.tile as tile
from concourse import mybir
from concourse.bass_utils import run_bass_kernel_spmd
from concourse.masks import make_identity

B, N, S = 8, 8192, 2048
D1, D2 = 128, 256
BN_EPS = 1e-5
REC_EPS = 1e-8
NT = N // 128          # 64 n-tiles
NSI = NT // 4          # 16 super-iterations (4 packed n-tiles each = 512 n)
NCH = N // 512         # 16 MLP chunks
CNT = float(B * N)     # BN population
F32 = mybir.dt.float32
F32R = mybir.dt.float32r
U32 = mybir.dt.uint32
AF = mybir.ActivationFunctionType
OP = mybir.AluOpType

_CACHE = {}


def _build_program():
    nc = bacc.Bacc("TRN2", target_bir_lowering=False, debug=False, num_devices=8)

    g1p_in = nc.dram_tensor("g1p", [NSI, 128, 128], F32, kind="ExternalInput").ap()
    g2r_in = nc.dram_tensor("g2r", [128, S], F32, kind="ExternalInput").ap()
    x1sq_in = nc.dram_tensor("x1sq", [NT, 128, 1], F32, kind="ExternalInput").ap()
    p2t_in = nc.dram_tensor("p2t", [S, D2], F32, kind="ExternalInput").ap()
    p1_in = nc.dram_tensor("p1", [D1, N], F32, kind="ExternalInput").ap()
    w0t_in = nc.dram_tensor("w0t", [128, 3, 2, 128], F32, kind="ExternalInput").ap()
    w1t_in = nc.dram_tensor("w1t", [128, 2, 2, 128], F32, kind="ExternalInput").ap()
    aff_in = nc.dram_tensor("aff", [2, 2, 128, 2], F32, kind="ExternalInput").ap()
    m1_in = nc.dram_tensor("m1", [128, 1], U32, kind="ExternalInput").ap()
    m2_in = nc.dram_tensor("m2", [128, 1], U32, kind="ExternalInput").ap()
    out_ext = nc.dram_tensor("out", [D2, N], F32, kind="ExternalOutput").ap()

    with tile.TileContext(nc) as tc:
        _emit(tc, nc, g1p_in, g2r_in, x1sq_in, p2t_in, p1_in, w0t_in, w1t_in,
              aff_in, out_ext, m1_in, m2_in)
    nc.compile()
    return nc


def _emit(tc, nc, g1p_in, g2r_in, x1sq_in, p2t_in, p1_in, w0t_in, w1t_in,
          aff_in, out_ext, m1_in, m2_in):
    from contextlib import ExitStack
    ctx = ExitStack()
    p1ctx = ExitStack()
    with ctx:
        singles = ctx.enter_context(tc.tile_pool(name="singles", bufs=1))
        small2 = ctx.enter_context(tc.tile_pool(name="small2", bufs=2))
        dpsum = ctx.enter_context(tc.tile_pool(name="dpsum", bufs=4, space="PSUM"))
        ipsum = ctx.enter_context(tc.tile_pool(name="ipsum", bufs=2, space="PSUM"))
        mpsum = ctx.enter_context(tc.tile_pool(name="mpsum", bufs=2, space="PSUM"))
        dram = ctx.enter_context(tc.tile_pool(name="dramb", bufs=1, space="DRAM"))
        res1 = p1ctx.enter_context(tc.tile_pool(name="res1", bufs=1))
        dsb_pool = p1ctx.enter_context(tc.tile_pool(name="dsb", bufs=5))
        small = p1ctx.enter_context(tc.tile_pool(name="small", bufs=4))
        gpool = p1ctx.enter_context(tc.tile_pool(name="gath", bufs=3))
        xkpool = p1ctx.enter_context(tc.tile_pool(name="xk", bufs=2))
        p1pool = p1ctx.enter_context(tc.tile_pool(name="p1c", bufs=2))
        dumpp = p1ctx.enter_context(tc.tile_pool(name="dump", bufs=2))

        # ---------- resident setup ----------
        m1t = res1.tile([128, 1], U32)
        nc.sync.dma_start(m1t[:], m1_in)
        m2t = res1.tile([128, 1], U32)
        nc.sync.dma_start(m2t[:], m2_in)
        # G2 split-fp32r prep: rows 32g+0..3 r2, +4..7 r2(dup), +8..11 e2.
        # Temporaries borrow the dsb pool's rotating [128, S] slots.
        g2raw = dsb_pool.tile([128, S], F32, tag="dsb", name="g2raw")
        nc.sync.dma_start(g2raw[:], g2r_in)
        g2q = res1.tile([128, S], F32R)
        nc.vector.tensor_copy(g2q[:], g2raw[:])
        g2dec = dsb_pool.tile([128, S], F32, tag="dsb", name="g2dec")
        nc.vector.tensor_copy(g2dec[:], g2q[:])
        g2e = dsb_pool.tile([128, S], F32, tag="dsb", name="g2e")
        nc.vector.tensor_sub(g2e[:], g2raw[:], g2dec[:])
        nc.vector.copy_predicated(g2raw[:], m2t[:].to_broadcast([128, S]), g2e[:])
        nc.vector.tensor_copy(g2q[:], g2raw[:])
        ident = res1.tile([128, 128], F32)
        make_identity(nc, ident[:])
        w0f = res1.tile([128, 3, 2, 128], F32)
        nc.sync.dma_start(w0f[:], w0t_in)
        w1f = res1.tile([128, 2, 2, 128], F32)
        nc.sync.dma_start(w1f[:], w1t_in)
        # fp32r-rounded copies for the MLP matmuls
        w0t = res1.tile([128, 3, 2, 128], F32R)
        nc.vector.tensor_copy(w0t[:], w0f[:])
        w1t = singles.tile([128, 2, 2, 128], F32R)
        nc.vector.tensor_copy(w1t[:], w1f[:])
        affs = singles.tile([128, 2, 2, 2], F32)  # [part, layer, chunk, g/b]
        for l in range(2):
            for mo in range(2):
                nc.sync.dma_start(affs[:, l, mo, :], aff_in[l, mo])
        epst = singles.tile([128, 1], F32)
        nc.vector.memset(epst[:], BN_EPS)
        y1sb = [singles.tile([128, N], F32, tag=f"y1sb{mo}", name=f"y1sb{mo}")
                for mo in range(2)]
        s1acc = singles.tile([128, 2, NCH], F32)
        s2acc = singles.tile([128, 2, NCH], F32)
        t1acc = singles.tile([128, 2, NCH], F32)
        t2acc = singles.tile([128, 2, NCH], F32)

        # ---------- phase 1 ----------
        for si in range(NSI):
            # G1 split-fp32r prep per super-tile
            g1sp = small.tile([128, 128], F32, tag="g1sp", name=f"g1sp{si}")
            nc.sync.dma_start(g1sp[:], g1p_in[si])
            g1q0 = small.tile([128, 128], F32R, tag="g1q0", name=f"g1q0{si}")
            nc.scalar.activation(out=g1q0[:], in_=g1sp[:], func=AF.Copy)
            g1dec = small.tile([128, 128], F32, tag="g1dec", name=f"g1dec{si}")
            nc.scalar.activation(out=g1dec[:], in_=g1q0[:], func=AF.Copy)
            g1e = small.tile([128, 128], F32, tag="g1e", name=f"g1e{si}")
            nc.gpsimd.tensor_sub(g1e[:], g1sp[:], g1dec[:])
            nc.vector.copy_predicated(g1sp[:], m1t[:].to_broadcast([128, 128]),
                                      g1e[:])
            g1q = small.tile([128, 128], F32R, tag="g1q", name=f"g1q{si}")
            nc.scalar.activation(out=g1q[:], in_=g1sp[:], func=AF.Copy)

            # 4 packed n-tiles: K=12 split-fp32r distance matmuls + drains
            dsbs = []
            for g in range(4):
                dsb = dsb_pool.tile([128, S], F32, tag="dsb", name=f"dsb{si}_{g}")
                dsbs.append(dsb)
            for sc in range(4):
                for g in range(4):
                    dp = dpsum.tile([128, 512], F32, tag="dp", name=f"dp{si}_{sc}_{g}")
                    nc.tensor.matmul(
                        out=dp[:],
                        lhsT=g1q[32 * g:32 * g + 12, :],
                        rhs=g2q[32 * g:32 * g + 12, sc * 512:(sc + 1) * 512],
                        start=True, stop=True, tile_position=(32 * g, 0),
                    )
                    dst = dsbs[g][:, sc * 512:(sc + 1) * 512]
                    if sc == 3:
                        nc.vector.tensor_copy(dst, dp[:])
                    else:
                        nc.scalar.activation(out=dst, in_=dp[:], func=AF.Copy)

            xk12 = xkpool.tile([128, 1024], F32R, tag="xk12", name=f"xk12_{si}")

            for g in range(4):
                t = 4 * si + g
                dsb = dsbs[g]
                x1sqt = small.tile([128, 1], F32, tag="x1sqt", name=f"x1sq{t}")
                nc.sync.dma_start(x1sqt[:], x1sq_in[t])

                m8 = small.tile([128, 8], F32, tag="m8", name=f"m8_{t}")
                nc.vector.max(out=m8[:], in_=dsb[:])
                idx8 = small.tile([128, 8], U32, tag="idx8", name=f"idx8_{t}")
                nc.vector.max_index(out=idx8[:], in_max=m8[:], in_values=dsb[:])

                # d_j = (x1sq + eps) - m_j   (eps pre-added on host)
                d3 = small.tile([128, 3], F32, tag="d3", name=f"d3_{t}")
                nc.scalar.activation(out=d3[:], in_=m8[:, 0:3], func=AF.Identity,
                                     scale=-1.0, bias=x1sqt[:])
                r3 = small.tile([128, 3], F32, tag="r3", name=f"r3_{t}")
                nc.vector.reciprocal(out=r3[:], in_=d3[:])
                rs = small.tile([128, 1], F32, tag="rs", name=f"rs_{t}")
                rdump = small.tile([128, 3], F32, tag="rdump", name=f"rdump{t}")
                nc.scalar.activation(out=rdump[:], in_=r3[:], func=AF.Copy,
                                     accum_out=rs[:])
                rsi_t = small.tile([128, 1], F32, tag="rsi", name=f"rsi_{t}")
                nc.vector.reciprocal(out=rsi_t[:], in_=rs[:])
                w3 = small.tile([128, 3], F32, tag="w3", name=f"w3_{t}")
                nc.gpsimd.tensor_tensor(out=w3[:], in0=r3[:],
                                        in1=rsi_t[:].to_broadcast([128, 3]),
                                        op=OP.mult)

                gj, dj = [], []
                for j in range(3):
                    gt = gpool.tile([128, D2], F32, tag=f"g{j}", name=f"g{t}_{j}")
                    nc.gpsimd.indirect_dma_start(
                        out=gt[:], out_offset=None, in_=p2t_in,
                        in_offset=bass.IndirectOffsetOnAxis(ap=idx8[:, j:j + 1],
                                                            axis=0),
                    )
                    gj.append(gt)
                for j in range(3):
                    d = small.tile([128, 128], F32, tag=f"diag{j}", name=f"dg{t}_{j}")
                    if j == 0:
                        nc.vector.tensor_tensor(
                            out=d[:], in0=ident[:],
                            in1=w3[:, j:j + 1].to_broadcast([128, 128]), op=OP.mult)
                    elif j == 1:
                        nc.scalar.activation(out=d[:], in_=ident[:], func=AF.Copy,
                                             scale=w3[:, j:j + 1])
                    else:
                        nc.gpsimd.tensor_tensor(
                            out=d[:], in0=ident[:],
                            in1=w3[:, j:j + 1].to_broadcast([128, 128]), op=OP.mult)
                    dj.append(d)

                it = ipsum.tile([128, 256], F32, tag="it", name=f"it{t}")
                for mo in range(2):
                    for j in range(3):
                        nc.tensor.matmul(
                            out=it[:, mo * 128:(mo + 1) * 128],
                            lhsT=gj[j][:, mo * 128:(mo + 1) * 128],
                            rhs=dj[j][:], start=(j == 0), stop=(j == 2),
                        )
                xkv = xk12[:].rearrange("p (k n) -> p k n", k=2)
                nc.scalar.activation(out=xkv[:, :, g * 128:(g + 1) * 128],
                                     in_=it[:].rearrange("p (k n) -> p k n", k=2),
                                     func=AF.Copy)

            # MLP layer 1 on chunk ci = si (fp32r)
            ci = si
            p1c = p1pool.tile([128, 512], F32, tag="p1c", name=f"p1c{ci}")
            nc.sync.dma_start(p1c[:], p1_in[:, ci * 512:(ci + 1) * 512])
            p1r = p1pool.tile([128, 512], F32R, tag="p1r", name=f"p1r{ci}")
            nc.scalar.activation(out=p1r[:], in_=p1c[:], func=AF.Copy)
            for mo in range(2):
                yp = mpsum.tile([128, 512], F32, tag="yp", name=f"yp{ci}_{mo}")
                nc.tensor.matmul(out=yp[:], lhsT=w0t[:, 0, mo, :], rhs=p1r[:],
                                 start=True, stop=False)
                nc.tensor.matmul(out=yp[:], lhsT=w0t[:, 1, mo, :],
                                 rhs=xk12[:, 0:512], start=False, stop=False)
                nc.tensor.matmul(out=yp[:], lhsT=w0t[:, 2, mo, :],
                                 rhs=xk12[:, 512:1024], start=False, stop=True)
                ysl = y1sb[mo][:, ci * 512:(ci + 1) * 512]
                nc.scalar.activation(out=ysl, in_=yp[:], func=AF.Copy,
                                     accum_out=s1acc[:, mo, ci:ci + 1])
                dump = dumpp.tile([128, 512], F32, tag="dump", name=f"du{ci}_{mo}")
                nc.scalar.activation(out=dump[:], in_=ysl, func=AF.Square,
                                     accum_out=s2acc[:, mo, ci:ci + 1])

        # ---------- free phase-1 SBUF, open phase-2/3 pools ----------
        p1ctx.close()
        ypool = ctx.enter_context(tc.tile_pool(name="ych", bufs=2))
        dumpp2 = ctx.enter_context(tc.tile_pool(name="dump2", bufs=2))
        y2sb = [ctx.enter_context(tc.tile_pool(name=f"y2sb{mo}", bufs=1)).tile(
                    [128, N], F32, name=f"y2sb{mo}") for mo in range(2)]

        # ---------- BN1 ----------
        scale0, shift0 = _bn_allreduce(tc, nc, singles, small2, dram, epst, affs,
                                       s1acc, s2acc, layer=0)

        # ---------- phase 2: normalize+relu y1 (fp32r), MLP layer 2 ----------
        for ci in range(NCH):
            y1n = []
            for mo in range(2):
                yn = ypool.tile([128, 512], F32R, tag=f"y1n{mo}", name=f"y1n{ci}_{mo}")
                src = y1sb[mo][:, ci * 512:(ci + 1) * 512]
                if mo == 0:
                    nc.scalar.activation(out=yn[:], in_=src, func=AF.Relu,
                                         bias=shift0[mo][:], scale=scale0[mo][:])
                else:
                    nc.vector.tensor_scalar(out=yn[:], in0=src,
                                            scalar1=scale0[mo][:],
                                            scalar2=shift0[mo][:],
                                            op0=OP.mult, op1=OP.add)
                    nc.vector.tensor_scalar_max(yn[:], yn[:], 0.0)
                y1n.append(yn)
            for mo in range(2):
                yp = mpsum.tile([128, 512], F32, tag="yp", name=f"y2p{ci}_{mo}")
                nc.tensor.matmul(out=yp[:], lhsT=w1t[:, 0, mo, :], rhs=y1n[0][:],
                                 start=True, stop=False)
                nc.tensor.matmul(out=yp[:], lhsT=w1t[:, 1, mo, :], rhs=y1n[1][:],
                                 start=False, stop=True)
                y2c = y2sb[mo][:, ci * 512:(ci + 1) * 512]
                if mo == 0:
                    nc.scalar.activation(out=y2c, in_=yp[:], func=AF.Copy,
                                         accum_out=t1acc[:, mo, ci:ci + 1])
                else:
                    nc.vector.tensor_scalar(out=y2c, in0=yp[:], scalar1=1.0,
                                            scalar2=0.0, op0=OP.mult, op1=OP.add,
                                            accum_out=t1acc[:, mo, ci:ci + 1])
                dump = dumpp2.tile([128, 512], F32, tag="dump2", name=f"d2{ci}_{mo}")
                nc.vector.scalar_tensor_tensor(out=dump[:], in0=y2c, scalar=1.0,
                                               in1=y2c, op0=OP.mult, op1=OP.mult,
                                               accum_out=t2acc[:, mo, ci:ci + 1])

        # ---------- BN2 ----------
        scale1, shift1 = _bn_allreduce(tc, nc, singles, small2, dram, epst, affs,
                                       t1acc, t2acc, layer=1)

        # ---------- phase 3 ----------
        for ci in range(NCH):
            for mo in range(2):
                y2c = y2sb[mo][:, ci * 512:(ci + 1) * 512]
                oc = ypool.tile([128, 512], F32, tag="oc", name=f"oc{ci}_{mo}")
                if mo == 0:
                    nc.scalar.activation(out=oc[:], in_=y2c, func=AF.Relu,
                                         bias=shift1[mo][:], scale=scale1[mo][:])
                else:
                    nc.vector.tensor_scalar(out=oc[:], in0=y2c,
                                            scalar1=scale1[mo][:],
                                            scalar2=shift1[mo][:],
                                            op0=OP.mult, op1=OP.add)
                    nc.vector.tensor_scalar_max(oc[:], oc[:], 0.0)
                nc.sync.dma_start(
                    out_ext[mo * 128:(mo + 1) * 128, ci * 512:(ci + 1) * 512], oc[:])


def _bn_allreduce(tc, nc, singles, small, dram, epst, affs, sacc, sqacc, layer):
    """Reduce per-chunk partials, AllReduce over 8 cores, return per-chunk
    (scale, shift) [128,1] tiles so BN+affine is y*scale + shift."""
    stats = singles.tile([128, 4], F32, tag=f"stats{layer}", name=f"stats{layer}")
    for mo in range(2):
        sdump = small.tile([128, NCH], F32, tag="sdump", name=f"sdump{layer}{mo}")
        nc.scalar.activation(out=sdump[:], in_=sacc[:, mo, :], func=AF.Copy,
                             accum_out=stats[:, mo:mo + 1])
        qdump = small.tile([128, NCH], F32, tag="qdump", name=f"qdump{layer}{mo}")
        nc.scalar.activation(out=qdump[:], in_=sqacc[:, mo, :], func=AF.Copy,
                             accum_out=stats[:, 2 + mo:3 + mo])
    inb = dram.tile([128, 4], F32, tag=f"arin{layer}", name=f"arin{layer}")
    outb = dram.tile([128, 4], F32, tag=f"arout{layer}", name=f"arout{layer}")
    nc.gpsimd.dma_start(inb[:], stats[:])
    nc.gpsimd.collective_compute(
        "AllReduce", mybir.AluOpType.add,
        replica_groups=[list(range(8))],
        ins=[inb.opt()], outs=[outb.opt()],
    )
    gstats = singles.tile([128, 4], F32, tag=f"gstats{layer}", name=f"gstats{layer}")
    nc.gpsimd.dma_start(gstats[:], outb[:])

    scale, shift = [], []
    for mo in range(2):
        mean = singles.tile([128, 1], F32, tag=f"mean{layer}{mo}",
                            name=f"mean{layer}{mo}")
        nc.gpsimd.tensor_scalar_mul(mean[:], gstats[:, mo:mo + 1], 1.0 / CNT)
        var = singles.tile([128, 1], F32, tag=f"var{layer}{mo}",
                           name=f"var{layer}{mo}")
        nc.gpsimd.tensor_scalar_mul(var[:], gstats[:, 2 + mo:3 + mo], 1.0 / CNT)
        msq = small.tile([128, 1], F32, tag="msq", name=f"msq{layer}{mo}")
        nc.gpsimd.tensor_mul(msq[:], mean[:], mean[:])
        nc.gpsimd.tensor_sub(var[:], var[:], msq[:])
        sd = singles.tile([128, 1], F32, tag=f"sd_{layer}{mo}", name=f"sd{layer}{mo}")
        nc.scalar.activation(out=sd[:], in_=var[:], func=AF.Sqrt, bias=epst[:])
        rsd = singles.tile([128, 1], F32, tag=f"rsd{layer}{mo}",
                           name=f"rsd{layer}{mo}")
        nc.vector.reciprocal(out=rsd[:], in_=sd[:])
        sc = singles.tile([128, 1], F32, tag=f"scale{layer}{mo}",
                          name=f"scale{layer}{mo}")
        nc.gpsimd.tensor_mul(sc[:], affs[:, layer, mo, 0:1], rsd[:])
        sh = singles.tile([128, 1], F32, tag=f"shift{layer}{mo}",
                          name=f"shift{layer}{mo}")
        nc.gpsimd.tensor_mul(sh[:], mean[:], sc[:])
        nc.gpsimd.tensor_sub(sh[:], affs[:, layer, mo, 1:2], sh[:])
        scale.append(sc)
        shift.append(sh)
    return scale, shift


def _prep_core_inputs(b, xyz1, xyz2, points1, points2, W0, W1, g0, beta0, g1, beta1):
    x1 = xyz1[b].astype(np.float32)          # [3, N]
    x2 = xyz2[b].astype(np.float32)          # [3, S]
    x1sq = ((x1 * x1).sum(0) + np.float32(REC_EPS)).astype(np.float32)
    x2sq = (x2 * x2).sum(0).astype(np.float32)
    g1m = np.concatenate([x1, np.ones((1, N), np.float32)], 0)   # [4, N]
    g1t = g1m.reshape(4, NT, 128).transpose(1, 0, 2)             # [NT, 4, 128]
    # packed split-fp32r layout: super-tile si holds tile 4si+g at partition
    # offset 32g, tripled into rows +0..3 (-> r), +4..7 (-> e), +8..11 (-> r)
    g1p = np.zeros((NSI, 128, 128), np.float32)
    for g in range(4):
        for blk in range(3):
            g1p[:, 32 * g + 4 * blk:32 * g + 4 * blk + 4, :] = g1t[g::4][:NSI]
    g2m = np.concatenate([2.0 * x2, -x2sq[None]], 0).astype(np.float32)  # [4, S]
    g2rep = np.zeros((128, S), np.float32)
    for g in range(4):
        for blk in range(3):
            g2rep[32 * g + 4 * blk:32 * g + 4 * blk + 4, :] = g2m
    pidx = np.arange(128) % 32
    m1 = ((pidx >= 4) & (pidx < 8)).astype(np.uint32).reshape(128, 1)
    m2 = ((pidx >= 8) & (pidx < 12)).astype(np.uint32).reshape(128, 1)
    w0t = np.ascontiguousarray(
        W0.T.reshape(3, 128, 2, 128).transpose(1, 0, 2, 3)).astype(np.float32)
    w1t = np.ascontiguousarray(
        W1.T.reshape(2, 128, 2, 128).transpose(1, 0, 2, 3)).astype(np.float32)
    aff = np.stack([
        np.stack([np.stack([g0.reshape(2, 128)[c], beta0.reshape(2, 128)[c]], -1)
                  for c in range(2)]),
        np.stack([np.stack([g1.reshape(2, 128)[c], beta1.reshape(2, 128)[c]], -1)
                  for c in range(2)]),
    ]).astype(np.float32)
    return {
        "g1p": g1p,
        "g2r": g2rep,
        "x1sq": np.ascontiguousarray(x1sq.reshape(NT, 128, 1)),
        "p2t": np.ascontiguousarray(points2[b].T).astype(np.float32),
        "p1": np.ascontiguousarray(points1[b]).astype(np.float32),
        "w0t": w0t,
        "w1t": w1t,
        "aff": aff,
        "m1": m1,
        "m2": m2,
    }


def kernel(xyz1, xyz2, points1, points2, W0, b0, g0, beta0, W1, b1, g1, beta1,
           **_ignored):
    # b0/b1 cancel exactly in training-mode BatchNorm (constant channel shift
    # moves y and its mean equally) so they are not used on device.
    if "nc" not in _CACHE:
        _CACHE["nc"] = _build_program()
    nc = _CACHE["nc"]
    in_maps = [
        _prep_core_inputs(b, np.asarray(xyz1), np.asarray(xyz2),
                          np.asarray(points1), np.asarray(points2),
                          np.asarray(W0), np.asarray(W1),
                          np.asarray(g0), np.asarray(beta0),
                          np.asarray(g1), np.asarray(beta1))
        for b in range(B)
    ]
    res = run_bass_kernel_spmd(nc, in_maps, list(range(8)))
    out = np.stack([res.results[c]["out"] for c in range(8)], axis=0)
    return out.astype(np.float32)
